# revision 19
# baseline (speedup 1.0000x reference)
"""Trainium2 Bass kernel for the CaputoEncoder model.

Model (see reference): feats = concat([caputo(x, 0.5), caputo(x, 1.0)], -1)
-> 2-layer LSTM(512) -> last timestep -> relu(linear).

Key simplifications:
  * caputo(x, 1.0) has coefficient 1/gamma(0) == 0 -> contributes zeros;
    only the alpha=0.5 branch matters, so only Wih0[:, :250] is ever used.
  * caputo(x, .5) = d*x - Wc@x (over time) == G @ x_b with G = diag(d) - Wc,
    host-precomputed; becomes a single matmul per batch.
  * tanh(g) = 2*sigmoid(2g) - 1 with the g-gate rows pre-scaled by 2 on the
    host, so one Sigmoid activation covers all four gates.

Sharding: pure data parallelism over batch (64 -> 8 per core, 8 cores).
All weights replicated; scatter/gather on host.

The two LSTM layers' scans are software-pipelined: layer 1 lags layer 0 by
one WIN-step window. Inside each For_i iteration we interleave one step of
layer 0 (window w) with one step of layer 1 (window w-1), so each layer's
serial elementwise chain hides under the other layer's 64 matmul+ldweights
pairs; this also keeps the PE continuously busy (HAM stays un-throttled).
Layer 1's input projection xw1 = A1 @ h0 + b1 is produced per-window as a
PE-efficient bulk matmul (f=WIN*PB) straight into SBUF.

On-core layout (hidden-major):
  hT, cT  : (128 part = hidden%128, cols = kchunk*8 + b)   [4*8=32 cols]
  gatesT  : (128 part = gate%128,  cols = gchunk*8 + b)    [16*8=128 cols]
  gate chunks host-permuted to [i, f, o, g]; sigmoid covers all 128 cols.
"""

import math
from contextlib import ExitStack

import numpy as np
import ml_dtypes

import concourse.bass as bass
import concourse.tile as tile
from concourse.tile import add_dep_helper
from concourse import mybir
from concourse.bass import ds
from concourse.bass_utils import run_bass_kernel_spmd

AF = mybir.ActivationFunctionType
OP = mybir.AluOpType
F32 = mybir.dt.float32
F32R = mybir.dt.float32r
BF16 = mybir.dt.bfloat16

B, T, N = 64, 512, 250
NP = 256          # n padded to 2 partition chunks
H = 512
G4 = 4 * H        # 2048
OUT = 1024
NCORES = 8
PB = B // NCORES  # 8 batches per core
WIN = 32          # scan steps per For_i iteration
NWIN = T // WIN

KC = H // 128     # 4 hidden chunks
GC = G4 // 128    # 16 gate chunks
NC2 = NP // 128   # 2 input chunks
CB = KC * PB      # 32 h/c columns


def _split_drain_waits(nc, max_waits=1):
    """This walrus build's CoreV3 codegen accepts at most one sem-wait per
    engine instruction (Drain/Matmult/... ISA structs have a single wait
    slot).  Move extra waits onto same-engine NoOps inserted immediately
    before the instruction — the engine blocks at the NoOp instead, which is
    semantically identical (same engine stream, same program point)."""
    for bb in nc.m.functions[0].blocks:
        insts = bb.instructions  # live list
        i = 0
        while i < len(insts):
            ins = insts[i]
            si = ins.sync_info
            if si is not None and len(si.on_wait) > max_waits:
                waits = list(si.on_wait)
                ins.sync_info = mybir.SyncInfo(
                    on_wait=waits[:max_waits], on_update=list(si.on_update)
                )
                for j, w in enumerate(waits[max_waits:]):
                    nop = mybir.InstNoOp(name=f"{ins.name}-wsplit{j}")
                    nop.engine = ins.engine
                    nop.sync_info = mybir.SyncInfo(on_wait=[w], on_update=[])
                    insts.insert(i, nop)
                    i += 1
            i += 1


class _LayerState:
    """Per-layer persistent scan state + pools."""

    def __init__(self, nc, tc, ctx, name, whh_sb):
        self.name = name
        self.whh_sb = whh_sb
        pool = ctx.enter_context(tc.tile_pool(name=f"{name}_state", bufs=1))
        self.h_cur = pool.tile([128, CB], BF16)
        self.c_cur = pool.tile([128, CB], F32)
        nc.vector.memset(self.h_cur[:], 0.0)
        nc.vector.memset(self.c_cur[:], 0.0)
        self.ps_pool = ctx.enter_context(
            tc.tile_pool(name=f"{name}_ps", bufs=3, space="PSUM")
        )
        self.ew_pool = ctx.enter_context(tc.tile_pool(name=f"{name}_ew", bufs=3))
        self.sigma_dep = None  # scheduler-only ACT-order edge
        self.hw_pool = ctx.enter_context(tc.tile_pool(name=f"{name}_hw", bufs=3))


def _gen_window_steps(nc, st, id_sb, xw_u_view, hwin, bias_sb=None):
    """Generator emitting one LSTM step per next() for a WIN-step window.

    st       : _LayerState
    id_sb    : (128, 128) f32 identity; xw enters PSUM via id.T @ xw so the
               gates never take a DVE add (sigmoid reads PSUM directly)
    xw_u_view: callable u -> AP (128, GC, PB) input contribution for step u
    hwin     : SBUF tile (128, KC*WIN*PB) bf16 to dump h_t into, or None
    """
    h_prev = [st.h_cur[:, kc * PB:(kc + 1) * PB] for kc in range(KC)]
    c_prev = st.c_cur
    for u in range(WIN):
        psum = st.ps_pool.tile([128, GC * PB], F32, tag=f"{st.name}ps")
        # xw[t] into psum first — no h dependency, runs in the PE bubble
        nc.tensor.matmul(
            psum.rearrange("p (g b) -> p g b", g=GC),
            id_sb[:],
            xw_u_view(u),
            start=True,
            stop=False,
        )
        if bias_sb is not None:
            nc.tensor.matmul(
                psum[:, :], id_sb[:], bias_sb[:, :], start=False, stop=False,
            )
        for kc in range(KC):
            for gc in range(GC):
                nc.tensor.matmul(
                    psum[:, gc * PB:(gc + 1) * PB],
                    st.whh_sb[:, kc, gc * 128:(gc + 1) * 128],
                    h_prev[kc],
                    start=False,
                    stop=(kc == KC - 1 and gc == GC - 1),
                )
        # one sigmoid over all 128 cols; g rows were pre-scaled by 2 so
        # tanh(g) = 2*sigmoid(2g) - 1
        acts = st.ew_pool.tile([128, GC * PB], F32, tag=f"{st.name}a")
        sig = nc.scalar.activation(acts[:], psum[:], AF.Sigmoid)
        if st.sigma_dep is not None:
            # scheduler-only edge: keep ACT FIFO order sigma/tanh alternating
            # between the two layers so neither tanh is gated behind the
            # other layer's matmul-waiting sigma
            add_dep_helper(sig.ins, st.sigma_dep, sync=False,
                           reason="act-order")
        gp = st.ew_pool.tile([128, CB], F32, tag=f"{st.name}gp")
        nc.vector.tensor_scalar(
            gp[:], acts[:, 3 * CB:], 2.0, 1.0, OP.mult, OP.subtract
        )
        # c = f*c + i*g ; h = o*tanh(c)
        ig = st.ew_pool.tile([128, CB], F32, tag=f"{st.name}ig")
        nc.vector.tensor_tensor(ig[:], acts[:, :CB], gp[:], OP.mult)
        fc = st.ew_pool.tile([128, CB], F32, tag=f"{st.name}fc")
        nc.vector.tensor_tensor(fc[:], acts[:, CB:2 * CB], c_prev[:], OP.mult)
        c_new = (
            st.c_cur if u == WIN - 1
            else st.hw_pool.tile([128, CB], F32, tag=f"{st.name}c")
        )
        nc.vector.tensor_tensor(c_new[:], fc[:], ig[:], OP.add)
        tc_t = st.ew_pool.tile([128, CB], F32, tag=f"{st.name}tc")
        tanh_inst = nc.scalar.activation(tc_t[:], c_new[:], AF.Tanh)
        acts_o = acts[:, 2 * CB:3 * CB].rearrange("p (k b) -> p k b", k=KC)
        tc_v = tc_t[:].rearrange("p (k b) -> p k b", k=KC)
        if hwin is not None:
            h_out = hwin.rearrange("p (k w b) -> p w k b", k=KC, w=WIN)[:, u]
        elif u == WIN - 1:
            h_out = st.h_cur[:].rearrange("p (k b) -> p k b", k=KC)
        else:
            h_tmp = st.hw_pool.tile([128, CB], BF16, tag=f"{st.name}h")
            h_out = h_tmp[:].rearrange("p (k b) -> p k b", k=KC)
        nc.vector.tensor_tensor(h_out, acts_o, tc_v, OP.mult)
        if hwin is not None and u == WIN - 1:
            nc.vector.tensor_copy(
                st.h_cur[:].rearrange("p (k b) -> p k b", k=KC), h_out
            )
        h_prev = [h_out[:, kc, :] for kc in range(KC)]
        c_prev = c_new
        yield tanh_inst.ins


QW = WIN // 4  # bulk xw1 quarter: 8 steps, f = QW*PB = 64


def _emit_bulk_xw1_quarter(nc, bulk_ps_pool, a1_sb, h0win, xw1sb, qi):
    """xw1 = A1 @ h0 for steps [qi*QW, (qi+1)*QW) of the current window
    (b1 is injected per-step by the bias id-matmul instead).

    Emitted right after L1's step qi*QW+QW-1 so the matmuls fill the PE
    bubble while L0's elementwise chain runs.  One 2-bank PSUM tile for
    all 16 gate chunks, drained by a single DVE copy."""
    qoff = qi * QW * PB
    psB = bulk_ps_pool.tile([128, GC * QW * PB], F32, tag="bps")
    for gc in range(GC):
        for kc in range(KC):
            nc.tensor.matmul(
                psB[:, gc * QW * PB:(gc + 1) * QW * PB],
                a1_sb[:, kc, gc * 128:(gc + 1) * 128],
                h0win[:, kc * WIN * PB + qoff:kc * WIN * PB + qoff + QW * PB],
                start=(kc == 0),
                stop=(kc == KC - 1),
            )
    dst = xw1sb[:].rearrange(
        "p (g w b) -> p g (w b)", g=GC, w=WIN
    )[:, :, qoff:qoff + QW * PB]
    nc.vector.tensor_copy(
        dst, psB[:].rearrange("p (g q) -> p g q", g=GC)
    )


def build_nc():
    nc = bass.Bass()

    x_in = nc.dram_tensor("x", [PB, T, NP], F32R, kind="ExternalInput")
    gt_in = nc.dram_tensor("gt", [KC, 128, T], F32R, kind="ExternalInput")
    a0_in = nc.dram_tensor("a0t", [NC2, 128, G4], BF16, kind="ExternalInput")
    b0_in = nc.dram_tensor("b0", [128, GC], F32, kind="ExternalInput")
    whh0_in = nc.dram_tensor("whh0t", [KC, 128, G4], BF16, kind="ExternalInput")
    a1_in = nc.dram_tensor("a1t", [KC, 128, G4], BF16, kind="ExternalInput")
    b1_in = nc.dram_tensor("b1", [128, GC], F32, kind="ExternalInput")
    whh1_in = nc.dram_tensor("whh1t", [KC, 128, G4], BF16, kind="ExternalInput")
    wout_in = nc.dram_tensor("woutt", [KC, 128, OUT], BF16, kind="ExternalInput")
    bout_in = nc.dram_tensor("boutr", [PB, OUT], F32, kind="ExternalInput")
    ident_in = nc.dram_tensor("ident", [128, 128], BF16, kind="ExternalInput")
    b1r_in = nc.dram_tensor("b1r", [128, GC * PB], BF16, kind="ExternalInput")
    out_ext = nc.dram_tensor("out", [PB, OUT], F32, kind="ExternalOutput")

    xw0_dram = nc.dram_tensor("xw0s", [GC, 128, PB, T], BF16)

    with tile.TileContext(nc) as tc:
        with ExitStack() as ctx:
            const_pool = ctx.enter_context(tc.tile_pool(name="consts", bufs=1))

            b0_sb = const_pool.tile([128, GC], F32)
            nc.sync.dma_start(b0_sb[:], b0_in[:, :])
            whh0_sb = const_pool.tile([128, KC, G4], BF16)
            nc.sync.dma_start(whh0_sb[:], whh0_in[:, :, :].rearrange("k p g -> p k g"))
            a1_sb = const_pool.tile([128, KC, G4], BF16)
            nc.sync.dma_start(a1_sb[:], a1_in[:, :, :].rearrange("k p g -> p k g"))
            b1_sb = const_pool.tile([128, GC], F32)
            nc.sync.dma_start(b1_sb[:], b1_in[:, :])
            whh1_sb = const_pool.tile([128, KC, G4], BF16)
            nc.sync.dma_start(whh1_sb[:], whh1_in[:, :, :].rearrange("k p g -> p k g"))
            id_sb = const_pool.tile([128, 128], BF16)
            nc.sync.dma_start(id_sb[:], ident_in[:, :])
            b1r_sb = const_pool.tile([128, GC * PB], BF16)
            nc.sync.dma_start(b1r_sb[:], b1r_in[:, :])

            # ---- phase A+B: featsT_b = x_bT @ G^T ; xw0 = A0 @ feats + b0 ----
            with tc.tile_pool(name="ab", bufs=1) as ab_pool, \
                 tc.tile_pool(name="abw", bufs=1) as abw_pool, \
                 tc.tile_pool(name="abf", bufs=2) as abf_pool, \
                 tc.tile_pool(name="abps", bufs=2, space="PSUM") as abps_pool:
                gt_sb = abw_pool.tile([128, KC, T], F32R)
                nc.sync.dma_start(
                    gt_sb[:], gt_in[:, :, :].rearrange("k p t -> p k t")
                )
                a0_sb = abw_pool.tile([128, NC2, G4], BF16)
                nc.sync.dma_start(
                    a0_sb[:], a0_in[:, :, :].rearrange("k p g -> p k g")
                )
                for b in range(PB):
                    x_sb = ab_pool.tile([128, KC, NP], F32R, tag="x")
                    nc.sync.dma_start(
                        x_sb[:], x_in[b].rearrange("(k p) n -> p k n", p=128)
                    )
                    fb = abf_pool.tile([128, NC2, T], BF16, tag="feats")
                    for mc in range(NC2):
                        psA = abps_pool.tile([128, T], F32, tag="psA")
                        for kc in range(KC):
                            nc.tensor.matmul(
                                psA[:],
                                x_sb[:, kc, mc * 128:(mc + 1) * 128],
                                gt_sb[:, kc, :],
                                start=(kc == 0),
                                stop=(kc == KC - 1),
                            )
                        nc.vector.tensor_copy(fb[:, mc, :], psA[:])
                    for gc in range(GC):
                        psB = abps_pool.tile([128, T], F32, tag="psB")
                        for kc in range(NC2):
                            nc.tensor.matmul(
                                psB[:],
                                a0_sb[:, kc, gc * 128:(gc + 1) * 128],
                                fb[:, kc, :],
                                start=(kc == 0),
                                stop=(kc == NC2 - 1),
                            )
                        xwb = ab_pool.tile([128, T], BF16, tag="xwb")
                        nc.scalar.activation(
                            xwb[:], psB[:], AF.Identity,
                            bias=b0_sb[:, gc:gc + 1],
                        )
                        nc.sync.dma_start(xw0_dram[gc, :, b, :], xwb[:])

            # ---- fused pipelined scan: L0(win w) + L1(win w-1) + bulk xw1(w) --
            st0 = _LayerState(nc, tc, ctx, "s0", whh0_sb)
            st1 = _LayerState(nc, tc, ctx, "s1", whh1_sb)
            pipe_pool = ctx.enter_context(tc.tile_pool(name="pipe", bufs=1))
            h0win = pipe_pool.tile([128, KC * WIN * PB], BF16)
            xw1sb = pipe_pool.tile([128, GC * WIN * PB], BF16)
            xw1_v = xw1sb[:].rearrange("p (g w b) -> p w g b", g=GC, w=WIN)

            with tc.tile_pool(name="win", bufs=2) as win_pool, \
                 tc.tile_pool(name="bulk_ps", bufs=1, space="PSUM") as bulk_ps_pool:

                def emit_win0(iw, run_l0, run_l1):
                    """One pipeline stage: L0 on window iw, L1 on window iw-1."""
                    if run_l0:
                        win = win_pool.tile([128, GC * PB * WIN], BF16, tag="win")
                        win_4d = win[:].rearrange(
                            "p (g b w) -> p g b w", g=GC, b=PB
                        )
                        for gcd in range(GC):
                            nc.sync.dma_start(
                                win_4d[:, gcd, :, :],
                                xw0_dram[gcd, :, :, ds(iw * WIN, WIN)],
                            )
                        g0 = _gen_window_steps(
                            nc, st0, id_sb,
                            lambda u: win_4d[:, :, :, u:u + 1],
                            h0win,
                        )
                    g1 = None
                    if run_l1:
                        g1 = _gen_window_steps(
                            nc, st1, id_sb, lambda u: xw1_v[:, u], None,
                            bias_sb=b1r_sb,
                        )
                    prev_t1 = None
                    for u in range(WIN):
                        t0 = None
                        if run_l0:
                            st0.sigma_dep = prev_t1
                            t0 = next(g0)
                        if g1 is not None:
                            st1.sigma_dep = t0
                            prev_t1 = next(g1)
                        # quarter-bulk xw1 after L1 has consumed its old cols
                        if run_l0 and u % QW == QW - 1:
                            _emit_bulk_xw1_quarter(
                                nc, bulk_ps_pool, a1_sb, h0win, xw1sb, u // QW,
                            )

                emit_win0(0, True, False)            # peel: L0 window 0
                with tc.For_i(
                    1, NWIN, 1, hint_engines=(mybir.EngineType.PE,)
                ) as iw:
                    emit_win0(iw, True, True)        # L0 win iw, L1 win iw-1
                emit_win0(NWIN, False, True)         # peel: L1 window 15

            # ---- phase F: out = relu(h1_last @ Wout.T + bout) ----
            with tc.tile_pool(name="f_ps", bufs=2, space="PSUM") as fps_pool, \
                 tc.tile_pool(name="f_o", bufs=1) as fo_pool:
                wout_sb = fo_pool.tile([128, KC, OUT], BF16)
                nc.sync.dma_start(
                    wout_sb[:], wout_in[:, :, :].rearrange("k p g -> p k g")
                )
                bout_sb = fo_pool.tile([PB, OUT], F32)
                nc.sync.dma_start(bout_sb[:], bout_in[:, :])
                out_sb = bout_sb
                for half in range(2):
                    psF = fps_pool.tile([PB, 512], F32, tag="psF")
                    for kc in range(KC):
                        nc.tensor.matmul(
                            psF[:],
                            st1.h_cur[:, kc * PB:(kc + 1) * PB],
                            wout_sb[:, kc, half * 512:(half + 1) * 512],
                            start=(kc == 0),
                            stop=(kc == KC - 1),
                        )
                    sl = slice(half * 512, (half + 1) * 512)
                    nc.vector.tensor_tensor(
                        out_sb[:, sl], psF[:], bout_sb[:, sl], OP.add
                    )
                    nc.vector.tensor_scalar_max(out_sb[:, sl], out_sb[:, sl], 0.0)
                nc.sync.dma_start(out_ext[:, :], out_sb[:])

    _split_drain_waits(nc)
    return nc


_NC_CACHE = None


def _get_nc():
    global _NC_CACHE
    if _NC_CACHE is None:
        _NC_CACHE = build_nc()
    return _NC_CACHE


def _prep_host(inputs):
    x = np.asarray(inputs["x"], dtype=np.float32)
    coef = 1.0 / math.gamma(0.5)
    t = np.arange(T, dtype=np.float64)
    diff = t[:, None] - t[None, :]
    W = np.where(diff > 0, (np.abs(diff) + 1e-6) ** -0.5, 0.0).astype(np.float32)
    d = (coef * W.sum(1)).astype(np.float32)
    G = (np.diag(d) - coef * W).astype(np.float32)  # feats_b = G @ x_b
    GT = np.ascontiguousarray(G.T).reshape(KC, 128, T)

    perm = np.concatenate([  # torch gate order i,f,g,o -> [i,f,o,g]
        np.arange(0, H), np.arange(H, 2 * H),
        np.arange(3 * H, 4 * H), np.arange(2 * H, 3 * H),
    ])
    # g rows scaled by 2: tanh(g) computed on-chip as 2*sigmoid(2g)-1
    gscale = np.ones((G4, 1), np.float32)
    gscale[3 * H:] = 2.0
    bf = ml_dtypes.bfloat16

    A0 = np.zeros((G4, NP), np.float32)
    A0[:, :N] = np.asarray(inputs["Wih0"], np.float32)[perm, :N] * gscale
    A0T = np.ascontiguousarray(A0.T).astype(bf).reshape(NC2, 128, G4)
    b0 = ((np.asarray(inputs["bih0"], np.float32)
           + np.asarray(inputs["bhh0"], np.float32))[perm] * gscale[:, 0])
    b0_t = np.ascontiguousarray(b0.reshape(GC, 128).T)
    Whh0T = np.ascontiguousarray(
        (np.asarray(inputs["Whh0"], np.float32)[perm] * gscale).T
    ).astype(bf).reshape(KC, 128, G4)

    A1T = np.ascontiguousarray(
        (np.asarray(inputs["Wih1"], np.float32)[perm] * gscale).T
    ).astype(bf).reshape(KC, 128, G4)
    b1 = ((np.asarray(inputs["bih1"], np.float32)
           + np.asarray(inputs["bhh1"], np.float32))[perm] * gscale[:, 0])
    b1_t = np.ascontiguousarray(b1.reshape(GC, 128).T)
    Whh1T = np.ascontiguousarray(
        (np.asarray(inputs["Whh1"], np.float32)[perm] * gscale).T
    ).astype(bf).reshape(KC, 128, G4)

    WoutT = np.ascontiguousarray(
        np.asarray(inputs["Wout"], np.float32).T
    ).astype(bf).reshape(KC, 128, OUT)
    bout_r = np.broadcast_to(
        np.asarray(inputs["bout"], np.float32), (PB, OUT)
    ).copy()

    xp = np.zeros((B, T, NP), np.float32)
    xp[:, :, :N] = x

    b1r = np.repeat(b1_t[:, :, None], PB, axis=2).reshape(128, GC * PB)
    shared = dict(
        gt=GT, a0t=A0T, b0=b0_t, whh0t=Whh0T, a1t=A1T, b1=b1_t,
        whh1t=Whh1T, woutt=WoutT, boutr=bout_r,
        ident=np.eye(128).astype(ml_dtypes.bfloat16),
        b1r=b1r.astype(ml_dtypes.bfloat16),
    )
    in_maps = []
    for c in range(NCORES):
        m = dict(shared)
        m["x"] = np.ascontiguousarray(xp[c * PB:(c + 1) * PB])
        in_maps.append(m)
    return in_maps


def kernel(**inputs):
    nc = _get_nc()
    in_maps = _prep_host(inputs)
    res = run_bass_kernel_spmd(nc, in_maps, core_ids=list(range(NCORES)))
    out = np.concatenate([r["out"] for r in res.results], axis=0)
    return out.astype(np.float32)


# revision 22
# speedup vs baseline: 1.0762x; 1.0762x over previous
"""Trainium2 Bass kernel for the CaputoEncoder model.

Model (see reference): feats = concat([caputo(x, 0.5), caputo(x, 1.0)], -1)
-> 2-layer LSTM(512) -> last timestep -> relu(linear).

Key simplifications:
  * caputo(x, 1.0) has coefficient 1/gamma(0) == 0 -> contributes zeros;
    only the alpha=0.5 branch matters, so only Wih0[:, :250] is ever used.
  * caputo(x, .5) = d*x - Wc@x (over time) == G @ x_b with G = diag(d) - Wc,
    host-precomputed; becomes a single matmul per batch.
  * tanh(g) = 2*sigmoid(2g) - 1 with the g-gate rows pre-scaled by 2 on the
    host, so one Sigmoid activation covers all four gates.

Sharding: pure data parallelism over batch (64 -> 8 per core, 8 cores).
All weights replicated; scatter/gather on host.

The two LSTM layers' scans are software-pipelined: layer 1 lags layer 0 by
one WIN-step window. Inside each For_i iteration we interleave one step of
layer 0 (window w) with one step of layer 1 (window w-1), so each layer's
serial elementwise chain hides under the other layer's 64 matmul+ldweights
pairs; this also keeps the PE continuously busy (HAM stays un-throttled).
Layer 1's input projection xw1 = A1 @ h0 + b1 is produced per-window as a
PE-efficient bulk matmul (f=WIN*PB) straight into SBUF.

On-core layout (hidden-major):
  hT, cT  : (128 part = hidden%128, cols = kchunk*8 + b)   [4*8=32 cols]
  gatesT  : (128 part = gate%128,  cols = gchunk*8 + b)    [16*8=128 cols]
  gate chunks host-permuted to [i, f, o, g]; sigmoid covers all 128 cols.
"""

import math
from contextlib import ExitStack

import numpy as np
import ml_dtypes

import concourse.bass as bass
import concourse.tile as tile
from concourse.tile import add_dep_helper
from concourse import mybir
from concourse.bass import ds
from concourse.bass_utils import run_bass_kernel_spmd

AF = mybir.ActivationFunctionType
OP = mybir.AluOpType
F32 = mybir.dt.float32
F32R = mybir.dt.float32r
BF16 = mybir.dt.bfloat16

B, T, N = 64, 512, 250
NP = 256          # n padded to 2 partition chunks
H = 512
G4 = 4 * H        # 2048
OUT = 1024
NCORES = 8
PB = B // NCORES  # 8 batches per core
WIN = 32          # scan steps per For_i iteration
NWIN = T // WIN

KC = H // 128     # 4 hidden chunks
GC = G4 // 128    # 16 gate chunks
NC2 = NP // 128   # 2 input chunks
CB = KC * PB      # 32 h/c columns


def _split_drain_waits(nc, max_waits=1):
    """This walrus build's CoreV3 codegen accepts at most one sem-wait per
    engine instruction (Drain/Matmult/... ISA structs have a single wait
    slot).  Move extra waits onto same-engine NoOps inserted immediately
    before the instruction — the engine blocks at the NoOp instead, which is
    semantically identical (same engine stream, same program point)."""
    for bb in nc.m.functions[0].blocks:
        insts = bb.instructions  # live list
        i = 0
        while i < len(insts):
            ins = insts[i]
            si = ins.sync_info
            if si is not None and len(si.on_wait) > max_waits:
                waits = list(si.on_wait)
                ins.sync_info = mybir.SyncInfo(
                    on_wait=waits[:max_waits], on_update=list(si.on_update)
                )
                for j, w in enumerate(waits[max_waits:]):
                    nop = mybir.InstNoOp(name=f"{ins.name}-wsplit{j}")
                    nop.engine = ins.engine
                    nop.sync_info = mybir.SyncInfo(on_wait=[w], on_update=[])
                    insts.insert(i, nop)
                    i += 1
            i += 1


class _LayerState:
    """Per-layer persistent scan state + pools."""

    def __init__(self, nc, tc, ctx, name, whh_sb):
        self.name = name
        self.whh_sb = whh_sb
        pool = ctx.enter_context(tc.tile_pool(name=f"{name}_state", bufs=1))
        self.h_cur = pool.tile([128, CB], BF16)
        self.c_cur = pool.tile([128, CB], F32)
        nc.vector.memset(self.h_cur[:], 0.0)
        nc.vector.memset(self.c_cur[:], 0.0)
        self.ps_pool = ctx.enter_context(
            tc.tile_pool(name=f"{name}_ps", bufs=3, space="PSUM")
        )
        self.ew_pool = ctx.enter_context(tc.tile_pool(name=f"{name}_ew", bufs=3))
        self.sigma_dep = None  # scheduler-only ACT-order edge
        self.hw_pool = ctx.enter_context(tc.tile_pool(name=f"{name}_hw", bufs=3))


def _gen_window_steps(nc, st, id_sb, xw_u_view, hwin, bias_sb=None):
    """Generator emitting one LSTM step per next() for a WIN-step window.

    st       : _LayerState
    id_sb    : (128, 128) f32 identity; xw enters PSUM via id.T @ xw so the
               gates never take a DVE add (sigmoid reads PSUM directly)
    xw_u_view: callable u -> AP (128, GC, PB) input contribution for step u
    hwin     : SBUF tile (128, KC*WIN*PB) bf16 to dump h_t into, or None
    """
    h_prev = [st.h_cur[:, kc * PB:(kc + 1) * PB] for kc in range(KC)]
    c_prev = st.c_cur
    for u in range(WIN):
        psum = st.ps_pool.tile([128, GC * PB], F32, tag=f"{st.name}ps")
        # xw[t] into psum first — no h dependency, runs in the PE bubble
        nc.tensor.matmul(
            psum.rearrange("p (g b) -> p g b", g=GC),
            id_sb[:],
            xw_u_view(u),
            start=True,
            stop=False,
        )
        if bias_sb is not None:
            nc.tensor.matmul(
                psum[:, :], id_sb[:], bias_sb[:, :], start=False, stop=False,
            )
        for kc in range(KC):
            for gc in range(GC):
                nc.tensor.matmul(
                    psum[:, gc * PB:(gc + 1) * PB],
                    st.whh_sb[:, kc, gc * 128:(gc + 1) * 128],
                    h_prev[kc],
                    start=False,
                    stop=(kc == KC - 1 and gc == GC - 1),
                )
        # one sigmoid over all 128 cols; g rows were pre-scaled by 2 so
        # tanh(g) = 2*sigmoid(2g) - 1
        acts = st.ew_pool.tile([128, GC * PB], F32, tag=f"{st.name}a")
        sig = nc.scalar.activation(acts[:], psum[:], AF.Sigmoid)
        if st.sigma_dep is not None:
            # scheduler-only edge: keep ACT FIFO order sigma/tanh alternating
            # between the two layers so neither tanh is gated behind the
            # other layer's matmul-waiting sigma
            add_dep_helper(sig.ins, st.sigma_dep, sync=False,
                           reason="act-order")
        gp = st.ew_pool.tile([128, CB], F32, tag=f"{st.name}gp")
        nc.vector.tensor_scalar(
            gp[:], acts[:, 3 * CB:], 2.0, 1.0, OP.mult, OP.subtract
        )
        # c = f*c + i*g ; h = o*tanh(c)
        ig = st.ew_pool.tile([128, CB], F32, tag=f"{st.name}ig")
        nc.vector.tensor_tensor(ig[:], acts[:, :CB], gp[:], OP.mult)
        fc = st.ew_pool.tile([128, CB], F32, tag=f"{st.name}fc")
        nc.vector.tensor_tensor(fc[:], acts[:, CB:2 * CB], c_prev[:], OP.mult)
        c_new = (
            st.c_cur if u == WIN - 1
            else st.hw_pool.tile([128, CB], F32, tag=f"{st.name}c")
        )
        nc.vector.tensor_tensor(c_new[:], fc[:], ig[:], OP.add)
        tc_t = st.ew_pool.tile([128, CB], F32, tag=f"{st.name}tc")
        tanh_inst = nc.scalar.activation(tc_t[:], c_new[:], AF.Tanh)
        acts_o = acts[:, 2 * CB:3 * CB].rearrange("p (k b) -> p k b", k=KC)
        tc_v = tc_t[:].rearrange("p (k b) -> p k b", k=KC)
        if hwin is not None:
            h_out = hwin.rearrange("p (k w b) -> p w k b", k=KC, w=WIN)[:, u]
        elif u == WIN - 1:
            h_out = st.h_cur[:].rearrange("p (k b) -> p k b", k=KC)
        else:
            h_tmp = st.hw_pool.tile([128, CB], BF16, tag=f"{st.name}h")
            h_out = h_tmp[:].rearrange("p (k b) -> p k b", k=KC)
        nc.vector.tensor_tensor(h_out, acts_o, tc_v, OP.mult)
        if hwin is not None and u == WIN - 1:
            nc.vector.tensor_copy(
                st.h_cur[:].rearrange("p (k b) -> p k b", k=KC), h_out
            )
        h_prev = [h_out[:, kc, :] for kc in range(KC)]
        c_prev = c_new
        yield tanh_inst.ins


QW = WIN // 4  # bulk xw1 quarter: 8 steps, f = QW*PB = 64


def _emit_bulk_xw1_quarter(nc, bulk_ps_pool, a1_sb, h0win, xw1sb, qi):
    """xw1 = A1 @ h0 for steps [qi*QW, (qi+1)*QW) of the current window
    (b1 is injected per-step by the bias id-matmul instead).

    Emitted right after L1's step qi*QW+QW-1 so the matmuls fill the PE
    bubble while L0's elementwise chain runs.  One 2-bank PSUM tile for
    all 16 gate chunks, drained by a single DVE copy."""
    qoff = qi * QW * PB
    psB = bulk_ps_pool.tile([128, GC * QW * PB], F32, tag="bps")
    for gc in range(GC):
        for kc in range(KC):
            nc.tensor.matmul(
                psB[:, gc * QW * PB:(gc + 1) * QW * PB],
                a1_sb[:, kc, gc * 128:(gc + 1) * 128],
                h0win[:, kc * WIN * PB + qoff:kc * WIN * PB + qoff + QW * PB],
                start=(kc == 0),
                stop=(kc == KC - 1),
            )
    dst = xw1sb[:].rearrange(
        "p (g w b) -> p g (w b)", g=GC, w=WIN
    )[:, :, qoff:qoff + QW * PB]
    nc.vector.tensor_copy(
        dst, psB[:].rearrange("p (g q) -> p g q", g=GC)
    )


def build_nc():
    nc = bass.Bass()

    x_in = nc.dram_tensor("x", [PB, T, NP], F32R, kind="ExternalInput")
    gt_in = nc.dram_tensor("gt", [KC, 128, T], F32R, kind="ExternalInput")
    a0_in = nc.dram_tensor("a0t", [NC2, 128, G4], BF16, kind="ExternalInput")
    b0_in = nc.dram_tensor("b0", [128, GC], F32, kind="ExternalInput")
    whh0_in = nc.dram_tensor("whh0t", [KC, 128, G4], BF16, kind="ExternalInput")
    a1_in = nc.dram_tensor("a1t", [KC, 128, G4], BF16, kind="ExternalInput")
    b1_in = nc.dram_tensor("b1", [128, GC], F32, kind="ExternalInput")
    whh1_in = nc.dram_tensor("whh1t", [KC, 128, G4], BF16, kind="ExternalInput")
    wout_in = nc.dram_tensor("woutt", [KC, 128, OUT], BF16, kind="ExternalInput")
    bout_in = nc.dram_tensor("boutr", [PB, OUT], F32, kind="ExternalInput")
    ident_in = nc.dram_tensor("ident", [128, 128], BF16, kind="ExternalInput")
    b1r_in = nc.dram_tensor("b1r", [128, GC * PB], BF16, kind="ExternalInput")
    out_ext = nc.dram_tensor("out", [PB, OUT], F32, kind="ExternalOutput")

    xw0_dram = nc.dram_tensor("xw0s", [GC, 128, PB, T], BF16)

    with tile.TileContext(nc) as tc:
        with ExitStack() as ctx:
            const_pool = ctx.enter_context(tc.tile_pool(name="consts", bufs=1))

            b0_sb = const_pool.tile([128, GC], F32)
            nc.sync.dma_start(b0_sb[:], b0_in[:, :])
            whh0_sb = const_pool.tile([128, KC, G4], BF16)
            nc.sync.dma_start(whh0_sb[:], whh0_in[:, :, :].rearrange("k p g -> p k g"))
            a1_sb = const_pool.tile([128, KC, G4], BF16)
            nc.sync.dma_start(a1_sb[:], a1_in[:, :, :].rearrange("k p g -> p k g"))
            b1_sb = const_pool.tile([128, GC], F32)
            nc.sync.dma_start(b1_sb[:], b1_in[:, :])
            whh1_sb = const_pool.tile([128, KC, G4], BF16)
            nc.sync.dma_start(whh1_sb[:], whh1_in[:, :, :].rearrange("k p g -> p k g"))
            id_sb = const_pool.tile([128, 128], BF16)
            nc.sync.dma_start(id_sb[:], ident_in[:, :])
            b1r_sb = const_pool.tile([128, GC * PB], BF16)
            nc.sync.dma_start(b1r_sb[:], b1r_in[:, :])

            # ---- phase A+B: featsT_b = x_bT @ G^T ; xw0 = A0 @ feats + b0 ----
            with tc.tile_pool(name="ab", bufs=2) as ab_pool, \
                 tc.tile_pool(name="abw", bufs=1) as abw_pool, \
                 tc.tile_pool(name="abf", bufs=1) as abf_pool, \
                 tc.tile_pool(name="abps", bufs=2, space="PSUM") as abps_pool:
                gt_sb = abw_pool.tile([128, KC, T], F32R)
                nc.sync.dma_start(
                    gt_sb[:], gt_in[:, :, :].rearrange("k p t -> p k t")
                )
                a0_sb = abw_pool.tile([128, NC2, G4], BF16)
                nc.sync.dma_start(
                    a0_sb[:], a0_in[:, :, :].rearrange("k p g -> p k g")
                )
                feats = []
                for b in range(PB):
                    x_sb = ab_pool.tile([128, KC, NP], F32R, tag="x")
                    nc.sync.dma_start(
                        x_sb[:], x_in[b].rearrange("(k p) n -> p k n", p=128)
                    )
                    fb = abf_pool.tile([128, NC2, T], BF16, tag=f"feats{b}")
                    for mc in range(NC2):
                        psA = abps_pool.tile([128, T], F32, tag="psA")
                        for kc in range(KC):
                            nc.tensor.matmul(
                                psA[:],
                                x_sb[:, kc, mc * 128:(mc + 1) * 128],
                                gt_sb[:, kc, :],
                                start=(kc == 0),
                                stop=(kc == KC - 1),
                            )
                        nc.vector.tensor_copy(fb[:, mc, :], psA[:])
                    feats.append(fb)
                dma_engs = [nc.sync, nc.scalar]
                for gc in range(GC):
                    for b in range(PB):
                        psB = abps_pool.tile([128, T], F32, tag="psB")
                        for kc in range(NC2):
                            nc.tensor.matmul(
                                psB[:],
                                a0_sb[:, kc, gc * 128:(gc + 1) * 128],
                                feats[b][:, kc, :],
                                start=(kc == 0),
                                stop=(kc == NC2 - 1),
                            )
                        xwb = ab_pool.tile([128, T], BF16, tag="xwb")
                        nc.scalar.activation(
                            xwb[:], psB[:], AF.Identity,
                            bias=b0_sb[:, gc:gc + 1],
                        )
                        dma_engs[b % 2].dma_start(
                            xw0_dram[gc, :, b, :], xwb[:]
                        )

            # ---- fused pipelined scan: L0(win w) + L1(win w-1) + bulk xw1(w) --
            st0 = _LayerState(nc, tc, ctx, "s0", whh0_sb)
            st1 = _LayerState(nc, tc, ctx, "s1", whh1_sb)
            pipe_pool = ctx.enter_context(tc.tile_pool(name="pipe", bufs=1))
            h0win = pipe_pool.tile([128, KC * WIN * PB], BF16)
            xw1sb = pipe_pool.tile([128, GC * WIN * PB], BF16)
            xw1_v = xw1sb[:].rearrange("p (g w b) -> p w g b", g=GC, w=WIN)

            with tc.tile_pool(name="win", bufs=2) as win_pool, \
                 tc.tile_pool(name="bulk_ps", bufs=1, space="PSUM") as bulk_ps_pool:

                def emit_win0(iw, run_l0, run_l1):
                    """One pipeline stage: L0 on window iw, L1 on window iw-1."""
                    if run_l0:
                        win = win_pool.tile([128, GC * PB * WIN], BF16, tag="win")
                        win_4d = win[:].rearrange(
                            "p (g b w) -> p g b w", g=GC, b=PB
                        )
                        wengs = [nc.sync, nc.scalar]
                        for gcd in range(GC):
                            wengs[gcd % 2].dma_start(
                                win_4d[:, gcd, :, :],
                                xw0_dram[gcd, :, :, ds(iw * WIN, WIN)],
                            )
                        g0 = _gen_window_steps(
                            nc, st0, id_sb,
                            lambda u: win_4d[:, :, :, u:u + 1],
                            h0win,
                        )
                    g1 = None
                    if run_l1:
                        g1 = _gen_window_steps(
                            nc, st1, id_sb, lambda u: xw1_v[:, u], None,
                            bias_sb=b1r_sb,
                        )
                    prev_t1 = None
                    for u in range(WIN):
                        t0 = None
                        if run_l0:
                            st0.sigma_dep = prev_t1
                            t0 = next(g0)
                        if g1 is not None:
                            st1.sigma_dep = t0
                            prev_t1 = next(g1)
                        # quarter-bulk xw1 after L1 has consumed its old cols
                        if run_l0 and u % QW == QW - 1:
                            _emit_bulk_xw1_quarter(
                                nc, bulk_ps_pool, a1_sb, h0win, xw1sb, u // QW,
                            )

                emit_win0(0, True, False)            # peel: L0 window 0
                with tc.For_i(
                    1, NWIN, 1, hint_engines=(mybir.EngineType.PE,)
                ) as iw:
                    emit_win0(iw, True, True)        # L0 win iw, L1 win iw-1
                emit_win0(NWIN, False, True)         # peel: L1 window 15

            # ---- phase F: out = relu(h1_last @ Wout.T + bout) ----
            with tc.tile_pool(name="f_ps", bufs=2, space="PSUM") as fps_pool, \
                 tc.tile_pool(name="f_o", bufs=1) as fo_pool:
                wout_sb = fo_pool.tile([128, KC, OUT], BF16)
                nc.sync.dma_start(
                    wout_sb[:], wout_in[:, :, :].rearrange("k p g -> p k g")
                )
                bout_sb = fo_pool.tile([PB, OUT], F32)
                nc.sync.dma_start(bout_sb[:], bout_in[:, :])
                out_sb = bout_sb
                for half in range(2):
                    psF = fps_pool.tile([PB, 512], F32, tag="psF")
                    for kc in range(KC):
                        nc.tensor.matmul(
                            psF[:],
                            st1.h_cur[:, kc * PB:(kc + 1) * PB],
                            wout_sb[:, kc, half * 512:(half + 1) * 512],
                            start=(kc == 0),
                            stop=(kc == KC - 1),
                        )
                    sl = slice(half * 512, (half + 1) * 512)
                    nc.vector.tensor_tensor(
                        out_sb[:, sl], psF[:], bout_sb[:, sl], OP.add
                    )
                    nc.vector.tensor_scalar_max(out_sb[:, sl], out_sb[:, sl], 0.0)
                nc.sync.dma_start(out_ext[:, :], out_sb[:])

    _split_drain_waits(nc)
    return nc


_NC_CACHE = None


def _get_nc():
    global _NC_CACHE
    if _NC_CACHE is None:
        _NC_CACHE = build_nc()
    return _NC_CACHE


def _prep_host(inputs):
    x = np.asarray(inputs["x"], dtype=np.float32)
    coef = 1.0 / math.gamma(0.5)
    t = np.arange(T, dtype=np.float64)
    diff = t[:, None] - t[None, :]
    W = np.where(diff > 0, (np.abs(diff) + 1e-6) ** -0.5, 0.0).astype(np.float32)
    d = (coef * W.sum(1)).astype(np.float32)
    G = (np.diag(d) - coef * W).astype(np.float32)  # feats_b = G @ x_b
    GT = np.ascontiguousarray(G.T).reshape(KC, 128, T)

    perm = np.concatenate([  # torch gate order i,f,g,o -> [i,f,o,g]
        np.arange(0, H), np.arange(H, 2 * H),
        np.arange(3 * H, 4 * H), np.arange(2 * H, 3 * H),
    ])
    # g rows scaled by 2: tanh(g) computed on-chip as 2*sigmoid(2g)-1
    gscale = np.ones((G4, 1), np.float32)
    gscale[3 * H:] = 2.0
    bf = ml_dtypes.bfloat16

    A0 = np.zeros((G4, NP), np.float32)
    A0[:, :N] = np.asarray(inputs["Wih0"], np.float32)[perm, :N] * gscale
    A0T = np.ascontiguousarray(A0.T).astype(bf).reshape(NC2, 128, G4)
    b0 = ((np.asarray(inputs["bih0"], np.float32)
           + np.asarray(inputs["bhh0"], np.float32))[perm] * gscale[:, 0])
    b0_t = np.ascontiguousarray(b0.reshape(GC, 128).T)
    Whh0T = np.ascontiguousarray(
        (np.asarray(inputs["Whh0"], np.float32)[perm] * gscale).T
    ).astype(bf).reshape(KC, 128, G4)

    A1T = np.ascontiguousarray(
        (np.asarray(inputs["Wih1"], np.float32)[perm] * gscale).T
    ).astype(bf).reshape(KC, 128, G4)
    b1 = ((np.asarray(inputs["bih1"], np.float32)
           + np.asarray(inputs["bhh1"], np.float32))[perm] * gscale[:, 0])
    b1_t = np.ascontiguousarray(b1.reshape(GC, 128).T)
    Whh1T = np.ascontiguousarray(
        (np.asarray(inputs["Whh1"], np.float32)[perm] * gscale).T
    ).astype(bf).reshape(KC, 128, G4)

    WoutT = np.ascontiguousarray(
        np.asarray(inputs["Wout"], np.float32).T
    ).astype(bf).reshape(KC, 128, OUT)
    bout_r = np.broadcast_to(
        np.asarray(inputs["bout"], np.float32), (PB, OUT)
    ).copy()

    xp = np.zeros((B, T, NP), np.float32)
    xp[:, :, :N] = x

    b1r = np.repeat(b1_t[:, :, None], PB, axis=2).reshape(128, GC * PB)
    shared = dict(
        gt=GT, a0t=A0T, b0=b0_t, whh0t=Whh0T, a1t=A1T, b1=b1_t,
        whh1t=Whh1T, woutt=WoutT, boutr=bout_r,
        ident=np.eye(128).astype(ml_dtypes.bfloat16),
        b1r=b1r.astype(ml_dtypes.bfloat16),
    )
    in_maps = []
    for c in range(NCORES):
        m = dict(shared)
        m["x"] = np.ascontiguousarray(xp[c * PB:(c + 1) * PB])
        in_maps.append(m)
    return in_maps


def kernel(**inputs):
    nc = _get_nc()
    in_maps = _prep_host(inputs)
    res = run_bass_kernel_spmd(nc, in_maps, core_ids=list(range(NCORES)))
    out = np.concatenate([r["out"] for r in res.results], axis=0)
    return out.astype(np.float32)


# revision 23
# speedup vs baseline: 1.0765x; 1.0003x over previous
"""Trainium2 Bass kernel for the CaputoEncoder model.

Model (see reference): feats = concat([caputo(x, 0.5), caputo(x, 1.0)], -1)
-> 2-layer LSTM(512) -> last timestep -> relu(linear).

Key simplifications:
  * caputo(x, 1.0) has coefficient 1/gamma(0) == 0 -> contributes zeros;
    only the alpha=0.5 branch matters, so only Wih0[:, :250] is ever used.
  * caputo(x, .5) = d*x - Wc@x (over time) == G @ x_b with G = diag(d) - Wc,
    host-precomputed; becomes a single matmul per batch.
  * tanh(g) = 2*sigmoid(2g) - 1 with the g-gate rows pre-scaled by 2 on the
    host, so one Sigmoid activation covers all four gates.

Sharding: pure data parallelism over batch (64 -> 8 per core, 8 cores).
All weights replicated; scatter/gather on host.

The two LSTM layers' scans are software-pipelined: layer 1 lags layer 0 by
one WIN-step window. Inside each For_i iteration we interleave one step of
layer 0 (window w) with one step of layer 1 (window w-1), so each layer's
serial elementwise chain hides under the other layer's 64 matmul+ldweights
pairs; this also keeps the PE continuously busy (HAM stays un-throttled).
Layer 1's input projection xw1 = A1 @ h0 + b1 is produced per-window as a
PE-efficient bulk matmul (f=WIN*PB) straight into SBUF.

On-core layout (hidden-major):
  hT, cT  : (128 part = hidden%128, cols = kchunk*8 + b)   [4*8=32 cols]
  gatesT  : (128 part = gate%128,  cols = gchunk*8 + b)    [16*8=128 cols]
  gate chunks host-permuted to [i, f, o, g]; sigmoid covers all 128 cols.
"""

import math
from contextlib import ExitStack

import numpy as np
import ml_dtypes

import concourse.bass as bass
import concourse.tile as tile
from concourse.tile import add_dep_helper
from concourse import mybir
from concourse.bass import ds
from concourse.bass_utils import run_bass_kernel_spmd

AF = mybir.ActivationFunctionType
OP = mybir.AluOpType
F32 = mybir.dt.float32
F32R = mybir.dt.float32r
BF16 = mybir.dt.bfloat16

B, T, N = 64, 512, 250
NP = 256          # n padded to 2 partition chunks
H = 512
G4 = 4 * H        # 2048
OUT = 1024
NCORES = 8
PB = B // NCORES  # 8 batches per core
WIN = 32          # scan steps per For_i iteration
NWIN = T // WIN

KC = H // 128     # 4 hidden chunks
GC = G4 // 128    # 16 gate chunks
NC2 = NP // 128   # 2 input chunks
CB = KC * PB      # 32 h/c columns


def _split_drain_waits(nc, max_waits=1):
    """This walrus build's CoreV3 codegen accepts at most one sem-wait per
    engine instruction (Drain/Matmult/... ISA structs have a single wait
    slot).  Move extra waits onto same-engine NoOps inserted immediately
    before the instruction — the engine blocks at the NoOp instead, which is
    semantically identical (same engine stream, same program point)."""
    for bb in nc.m.functions[0].blocks:
        insts = bb.instructions  # live list
        i = 0
        while i < len(insts):
            ins = insts[i]
            si = ins.sync_info
            if si is not None and len(si.on_wait) > max_waits:
                waits = list(si.on_wait)
                ins.sync_info = mybir.SyncInfo(
                    on_wait=waits[:max_waits], on_update=list(si.on_update)
                )
                for j, w in enumerate(waits[max_waits:]):
                    nop = mybir.InstNoOp(name=f"{ins.name}-wsplit{j}")
                    nop.engine = ins.engine
                    nop.sync_info = mybir.SyncInfo(on_wait=[w], on_update=[])
                    insts.insert(i, nop)
                    i += 1
            i += 1


class _LayerState:
    """Per-layer persistent scan state + pools."""

    def __init__(self, nc, tc, ctx, name, whh_sb):
        self.name = name
        self.whh_sb = whh_sb
        pool = ctx.enter_context(tc.tile_pool(name=f"{name}_state", bufs=1))
        self.h_cur = pool.tile([128, CB], BF16)
        self.c_cur = pool.tile([128, CB], F32)
        nc.vector.memset(self.h_cur[:], 0.0)
        nc.vector.memset(self.c_cur[:], 0.0)
        self.ps_pool = ctx.enter_context(
            tc.tile_pool(name=f"{name}_ps", bufs=3, space="PSUM")
        )
        self.ew_pool = ctx.enter_context(tc.tile_pool(name=f"{name}_ew", bufs=3))
        self.sigma_dep = None  # scheduler-only ACT-order edge
        self.hw_pool = ctx.enter_context(tc.tile_pool(name=f"{name}_hw", bufs=3))


def _gen_window_steps(nc, st, id_sb, xw_u_view, hwin, bias_sb=None):
    """Generator emitting one LSTM step per next() for a WIN-step window.

    st       : _LayerState
    id_sb    : (128, 128) f32 identity; xw enters PSUM via id.T @ xw so the
               gates never take a DVE add (sigmoid reads PSUM directly)
    xw_u_view: callable u -> AP (128, GC, PB) input contribution for step u
    hwin     : SBUF tile (128, KC*WIN*PB) bf16 to dump h_t into, or None
    """
    h_prev = [st.h_cur[:, kc * PB:(kc + 1) * PB] for kc in range(KC)]
    c_prev = st.c_cur
    for u in range(WIN):
        psum = st.ps_pool.tile([128, GC * PB], F32, tag=f"{st.name}ps")
        # xw[t] into psum first — no h dependency, runs in the PE bubble
        nc.tensor.matmul(
            psum.rearrange("p (g b) -> p g b", g=GC),
            id_sb[:],
            xw_u_view(u),
            start=True,
            stop=False,
        )
        if bias_sb is not None:
            nc.tensor.matmul(
                psum[:, :], id_sb[:], bias_sb[:, :], start=False, stop=False,
            )
        for gc in range(GC):
            for kc in range(KC):
                nc.tensor.matmul(
                    psum[:, gc * PB:(gc + 1) * PB],
                    st.whh_sb[:, kc, gc * 128:(gc + 1) * 128],
                    h_prev[kc],
                    start=False,
                    stop=(gc == GC - 1 and kc == KC - 1),
                )
        # one sigmoid over all 128 cols; g rows were pre-scaled by 2 so
        # tanh(g) = 2*sigmoid(2g) - 1
        acts = st.ew_pool.tile([128, GC * PB], F32, tag=f"{st.name}a")
        sig = nc.scalar.activation(acts[:], psum[:], AF.Sigmoid)
        if st.sigma_dep is not None:
            # scheduler-only edge: keep ACT FIFO order sigma/tanh alternating
            # between the two layers so neither tanh is gated behind the
            # other layer's matmul-waiting sigma
            add_dep_helper(sig.ins, st.sigma_dep, sync=False,
                           reason="act-order")
        gp = st.ew_pool.tile([128, CB], F32, tag=f"{st.name}gp")
        nc.vector.tensor_scalar(
            gp[:], acts[:, 3 * CB:], 2.0, 1.0, OP.mult, OP.subtract
        )
        # c = f*c + i*g ; h = o*tanh(c)
        ig = st.ew_pool.tile([128, CB], F32, tag=f"{st.name}ig")
        nc.vector.tensor_tensor(ig[:], acts[:, :CB], gp[:], OP.mult)
        fc = st.ew_pool.tile([128, CB], F32, tag=f"{st.name}fc")
        nc.vector.tensor_tensor(fc[:], acts[:, CB:2 * CB], c_prev[:], OP.mult)
        c_new = (
            st.c_cur if u == WIN - 1
            else st.hw_pool.tile([128, CB], F32, tag=f"{st.name}c")
        )
        nc.vector.tensor_tensor(c_new[:], fc[:], ig[:], OP.add)
        tc_t = st.ew_pool.tile([128, CB], F32, tag=f"{st.name}tc")
        tanh_inst = nc.scalar.activation(tc_t[:], c_new[:], AF.Tanh)
        acts_o = acts[:, 2 * CB:3 * CB].rearrange("p (k b) -> p k b", k=KC)
        tc_v = tc_t[:].rearrange("p (k b) -> p k b", k=KC)
        if hwin is not None:
            h_out = hwin.rearrange("p (k w b) -> p w k b", k=KC, w=WIN)[:, u]
        elif u == WIN - 1:
            h_out = st.h_cur[:].rearrange("p (k b) -> p k b", k=KC)
        else:
            h_tmp = st.hw_pool.tile([128, CB], BF16, tag=f"{st.name}h")
            h_out = h_tmp[:].rearrange("p (k b) -> p k b", k=KC)
        nc.vector.tensor_tensor(h_out, acts_o, tc_v, OP.mult)
        if hwin is not None and u == WIN - 1:
            nc.vector.tensor_copy(
                st.h_cur[:].rearrange("p (k b) -> p k b", k=KC), h_out
            )
        h_prev = [h_out[:, kc, :] for kc in range(KC)]
        c_prev = c_new
        yield tanh_inst.ins


QW = WIN // 4  # bulk xw1 quarter: 8 steps, f = QW*PB = 64


def _emit_bulk_xw1_quarter(nc, bulk_ps_pool, a1_sb, h0win, xw1sb, qi):
    """xw1 = A1 @ h0 for steps [qi*QW, (qi+1)*QW) of the current window
    (b1 is injected per-step by the bias id-matmul instead).

    Emitted right after L1's step qi*QW+QW-1 so the matmuls fill the PE
    bubble while L0's elementwise chain runs.  One 2-bank PSUM tile for
    all 16 gate chunks, drained by a single DVE copy."""
    qoff = qi * QW * PB
    psB = bulk_ps_pool.tile([128, GC * QW * PB], F32, tag="bps")
    for gc in range(GC):
        for kc in range(KC):
            nc.tensor.matmul(
                psB[:, gc * QW * PB:(gc + 1) * QW * PB],
                a1_sb[:, kc, gc * 128:(gc + 1) * 128],
                h0win[:, kc * WIN * PB + qoff:kc * WIN * PB + qoff + QW * PB],
                start=(kc == 0),
                stop=(kc == KC - 1),
            )
    dst = xw1sb[:].rearrange(
        "p (g w b) -> p g (w b)", g=GC, w=WIN
    )[:, :, qoff:qoff + QW * PB]
    nc.vector.tensor_copy(
        dst, psB[:].rearrange("p (g q) -> p g q", g=GC)
    )


def build_nc():
    nc = bass.Bass()

    x_in = nc.dram_tensor("x", [PB, T, NP], F32R, kind="ExternalInput")
    gt_in = nc.dram_tensor("gt", [KC, 128, T], F32R, kind="ExternalInput")
    a0_in = nc.dram_tensor("a0t", [NC2, 128, G4], BF16, kind="ExternalInput")
    b0_in = nc.dram_tensor("b0", [128, GC], F32, kind="ExternalInput")
    whh0_in = nc.dram_tensor("whh0t", [KC, 128, G4], BF16, kind="ExternalInput")
    a1_in = nc.dram_tensor("a1t", [KC, 128, G4], BF16, kind="ExternalInput")
    b1_in = nc.dram_tensor("b1", [128, GC], F32, kind="ExternalInput")
    whh1_in = nc.dram_tensor("whh1t", [KC, 128, G4], BF16, kind="ExternalInput")
    wout_in = nc.dram_tensor("woutt", [KC, 128, OUT], BF16, kind="ExternalInput")
    bout_in = nc.dram_tensor("boutr", [PB, OUT], F32, kind="ExternalInput")
    ident_in = nc.dram_tensor("ident", [128, 128], BF16, kind="ExternalInput")
    b1r_in = nc.dram_tensor("b1r", [128, GC * PB], BF16, kind="ExternalInput")
    out_ext = nc.dram_tensor("out", [PB, OUT], F32, kind="ExternalOutput")

    xw0_dram = nc.dram_tensor("xw0s", [GC, 128, PB, T], BF16)

    with tile.TileContext(nc) as tc:
        with ExitStack() as ctx:
            const_pool = ctx.enter_context(tc.tile_pool(name="consts", bufs=1))

            b0_sb = const_pool.tile([128, GC], F32)
            nc.sync.dma_start(b0_sb[:], b0_in[:, :])
            whh0_sb = const_pool.tile([128, KC, G4], BF16)
            nc.sync.dma_start(whh0_sb[:], whh0_in[:, :, :].rearrange("k p g -> p k g"))
            a1_sb = const_pool.tile([128, KC, G4], BF16)
            nc.sync.dma_start(a1_sb[:], a1_in[:, :, :].rearrange("k p g -> p k g"))
            b1_sb = const_pool.tile([128, GC], F32)
            nc.sync.dma_start(b1_sb[:], b1_in[:, :])
            whh1_sb = const_pool.tile([128, KC, G4], BF16)
            nc.sync.dma_start(whh1_sb[:], whh1_in[:, :, :].rearrange("k p g -> p k g"))
            id_sb = const_pool.tile([128, 128], BF16)
            nc.sync.dma_start(id_sb[:], ident_in[:, :])
            b1r_sb = const_pool.tile([128, GC * PB], BF16)
            nc.sync.dma_start(b1r_sb[:], b1r_in[:, :])

            # ---- phase A+B: featsT_b = x_bT @ G^T ; xw0 = A0 @ feats + b0 ----
            with tc.tile_pool(name="ab", bufs=2) as ab_pool, \
                 tc.tile_pool(name="abw", bufs=1) as abw_pool, \
                 tc.tile_pool(name="abf", bufs=1) as abf_pool, \
                 tc.tile_pool(name="abps", bufs=2, space="PSUM") as abps_pool:
                gt_sb = abw_pool.tile([128, KC, T], F32R)
                nc.sync.dma_start(
                    gt_sb[:], gt_in[:, :, :].rearrange("k p t -> p k t")
                )
                a0_sb = abw_pool.tile([128, NC2, G4], BF16)
                nc.sync.dma_start(
                    a0_sb[:], a0_in[:, :, :].rearrange("k p g -> p k g")
                )
                feats = []
                for b in range(PB):
                    x_sb = ab_pool.tile([128, KC, NP], F32R, tag="x")
                    nc.sync.dma_start(
                        x_sb[:], x_in[b].rearrange("(k p) n -> p k n", p=128)
                    )
                    fb = abf_pool.tile([128, NC2, T], BF16, tag=f"feats{b}")
                    for mc in range(NC2):
                        psA = abps_pool.tile([128, T], F32, tag="psA")
                        for kc in range(KC):
                            nc.tensor.matmul(
                                psA[:],
                                x_sb[:, kc, mc * 128:(mc + 1) * 128],
                                gt_sb[:, kc, :],
                                start=(kc == 0),
                                stop=(kc == KC - 1),
                            )
                        nc.vector.tensor_copy(fb[:, mc, :], psA[:])
                    feats.append(fb)
                dma_engs = [nc.sync, nc.scalar]
                for gc in range(GC):
                    for b in range(PB):
                        psB = abps_pool.tile([128, T], F32, tag="psB")
                        for kc in range(NC2):
                            nc.tensor.matmul(
                                psB[:],
                                a0_sb[:, kc, gc * 128:(gc + 1) * 128],
                                feats[b][:, kc, :],
                                start=(kc == 0),
                                stop=(kc == NC2 - 1),
                            )
                        xwb = ab_pool.tile([128, T], BF16, tag="xwb")
                        nc.scalar.activation(
                            xwb[:], psB[:], AF.Identity,
                            bias=b0_sb[:, gc:gc + 1],
                        )
                        dma_engs[b % 2].dma_start(
                            xw0_dram[gc, :, b, :], xwb[:]
                        )

            # ---- fused pipelined scan: L0(win w) + L1(win w-1) + bulk xw1(w) --
            st0 = _LayerState(nc, tc, ctx, "s0", whh0_sb)
            st1 = _LayerState(nc, tc, ctx, "s1", whh1_sb)
            pipe_pool = ctx.enter_context(tc.tile_pool(name="pipe", bufs=1))
            h0win = pipe_pool.tile([128, KC * WIN * PB], BF16)
            xw1sb = pipe_pool.tile([128, GC * WIN * PB], BF16)
            xw1_v = xw1sb[:].rearrange("p (g w b) -> p w g b", g=GC, w=WIN)

            with tc.tile_pool(name="win", bufs=2) as win_pool, \
                 tc.tile_pool(name="bulk_ps", bufs=1, space="PSUM") as bulk_ps_pool:

                def emit_win0(iw, run_l0, run_l1):
                    """One pipeline stage: L0 on window iw, L1 on window iw-1."""
                    if run_l0:
                        win = win_pool.tile([128, GC * PB * WIN], BF16, tag="win")
                        win_4d = win[:].rearrange(
                            "p (g b w) -> p g b w", g=GC, b=PB
                        )
                        wengs = [nc.sync, nc.scalar]
                        for bd in range(PB):
                            wengs[bd % 2].dma_start(
                                win_4d[:, :, bd, :],
                                xw0_dram[:, :, bd, ds(iw * WIN, WIN)]
                                .rearrange("g p w -> p g w"),
                            )
                        g0 = _gen_window_steps(
                            nc, st0, id_sb,
                            lambda u: win_4d[:, :, :, u:u + 1],
                            h0win,
                        )
                    g1 = None
                    if run_l1:
                        g1 = _gen_window_steps(
                            nc, st1, id_sb, lambda u: xw1_v[:, u], None,
                            bias_sb=b1r_sb,
                        )
                    prev_t1 = None
                    for u in range(WIN):
                        t0 = None
                        if run_l0:
                            st0.sigma_dep = prev_t1
                            t0 = next(g0)
                        if g1 is not None:
                            st1.sigma_dep = t0
                            prev_t1 = next(g1)
                        # quarter-bulk xw1 after L1 has consumed its old cols
                        if run_l0 and u % QW == QW - 1:
                            _emit_bulk_xw1_quarter(
                                nc, bulk_ps_pool, a1_sb, h0win, xw1sb, u // QW,
                            )

                emit_win0(0, True, False)            # peel: L0 window 0
                with tc.For_i(
                    1, NWIN, 1, hint_engines=(mybir.EngineType.PE,)
                ) as iw:
                    emit_win0(iw, True, True)        # L0 win iw, L1 win iw-1
                emit_win0(NWIN, False, True)         # peel: L1 window 15

            # ---- phase F: out = relu(h1_last @ Wout.T + bout) ----
            with tc.tile_pool(name="f_ps", bufs=2, space="PSUM") as fps_pool, \
                 tc.tile_pool(name="f_o", bufs=1) as fo_pool:
                wout_sb = fo_pool.tile([128, KC, OUT], BF16)
                nc.sync.dma_start(
                    wout_sb[:], wout_in[:, :, :].rearrange("k p g -> p k g")
                )
                bout_sb = fo_pool.tile([PB, OUT], F32)
                nc.sync.dma_start(bout_sb[:], bout_in[:, :])
                out_sb = bout_sb
                for half in range(2):
                    psF = fps_pool.tile([PB, 512], F32, tag="psF")
                    for kc in range(KC):
                        nc.tensor.matmul(
                            psF[:],
                            st1.h_cur[:, kc * PB:(kc + 1) * PB],
                            wout_sb[:, kc, half * 512:(half + 1) * 512],
                            start=(kc == 0),
                            stop=(kc == KC - 1),
                        )
                    sl = slice(half * 512, (half + 1) * 512)
                    nc.vector.tensor_tensor(
                        out_sb[:, sl], psF[:], bout_sb[:, sl], OP.add
                    )
                    nc.vector.tensor_scalar_max(out_sb[:, sl], out_sb[:, sl], 0.0)
                nc.sync.dma_start(out_ext[:, :], out_sb[:])

    _split_drain_waits(nc)
    return nc


_NC_CACHE = None


def _get_nc():
    global _NC_CACHE
    if _NC_CACHE is None:
        _NC_CACHE = build_nc()
    return _NC_CACHE


def _prep_host(inputs):
    x = np.asarray(inputs["x"], dtype=np.float32)
    coef = 1.0 / math.gamma(0.5)
    t = np.arange(T, dtype=np.float64)
    diff = t[:, None] - t[None, :]
    W = np.where(diff > 0, (np.abs(diff) + 1e-6) ** -0.5, 0.0).astype(np.float32)
    d = (coef * W.sum(1)).astype(np.float32)
    G = (np.diag(d) - coef * W).astype(np.float32)  # feats_b = G @ x_b
    GT = np.ascontiguousarray(G.T).reshape(KC, 128, T)

    perm = np.concatenate([  # torch gate order i,f,g,o -> [i,f,o,g]
        np.arange(0, H), np.arange(H, 2 * H),
        np.arange(3 * H, 4 * H), np.arange(2 * H, 3 * H),
    ])
    # g rows scaled by 2: tanh(g) computed on-chip as 2*sigmoid(2g)-1
    gscale = np.ones((G4, 1), np.float32)
    gscale[3 * H:] = 2.0
    bf = ml_dtypes.bfloat16

    A0 = np.zeros((G4, NP), np.float32)
    A0[:, :N] = np.asarray(inputs["Wih0"], np.float32)[perm, :N] * gscale
    A0T = np.ascontiguousarray(A0.T).astype(bf).reshape(NC2, 128, G4)
    b0 = ((np.asarray(inputs["bih0"], np.float32)
           + np.asarray(inputs["bhh0"], np.float32))[perm] * gscale[:, 0])
    b0_t = np.ascontiguousarray(b0.reshape(GC, 128).T)
    Whh0T = np.ascontiguousarray(
        (np.asarray(inputs["Whh0"], np.float32)[perm] * gscale).T
    ).astype(bf).reshape(KC, 128, G4)

    A1T = np.ascontiguousarray(
        (np.asarray(inputs["Wih1"], np.float32)[perm] * gscale).T
    ).astype(bf).reshape(KC, 128, G4)
    b1 = ((np.asarray(inputs["bih1"], np.float32)
           + np.asarray(inputs["bhh1"], np.float32))[perm] * gscale[:, 0])
    b1_t = np.ascontiguousarray(b1.reshape(GC, 128).T)
    Whh1T = np.ascontiguousarray(
        (np.asarray(inputs["Whh1"], np.float32)[perm] * gscale).T
    ).astype(bf).reshape(KC, 128, G4)

    WoutT = np.ascontiguousarray(
        np.asarray(inputs["Wout"], np.float32).T
    ).astype(bf).reshape(KC, 128, OUT)
    bout_r = np.broadcast_to(
        np.asarray(inputs["bout"], np.float32), (PB, OUT)
    ).copy()

    xp = np.zeros((B, T, NP), np.float32)
    xp[:, :, :N] = x

    b1r = np.repeat(b1_t[:, :, None], PB, axis=2).reshape(128, GC * PB)
    shared = dict(
        gt=GT, a0t=A0T, b0=b0_t, whh0t=Whh0T, a1t=A1T, b1=b1_t,
        whh1t=Whh1T, woutt=WoutT, boutr=bout_r,
        ident=np.eye(128).astype(ml_dtypes.bfloat16),
        b1r=b1r.astype(ml_dtypes.bfloat16),
    )
    in_maps = []
    for c in range(NCORES):
        m = dict(shared)
        m["x"] = np.ascontiguousarray(xp[c * PB:(c + 1) * PB])
        in_maps.append(m)
    return in_maps


def kernel(**inputs):
    nc = _get_nc()
    in_maps = _prep_host(inputs)
    res = run_bass_kernel_spmd(nc, in_maps, core_ids=list(range(NCORES)))
    out = np.concatenate([r["out"] for r in res.results], axis=0)
    return out.astype(np.float32)


# revision 24
# speedup vs baseline: 1.0770x; 1.0005x over previous
"""Trainium2 Bass kernel for the CaputoEncoder model.

Model (see reference): feats = concat([caputo(x, 0.5), caputo(x, 1.0)], -1)
-> 2-layer LSTM(512) -> last timestep -> relu(linear).

Key simplifications:
  * caputo(x, 1.0) has coefficient 1/gamma(0) == 0 -> contributes zeros;
    only the alpha=0.5 branch matters, so only Wih0[:, :250] is ever used.
  * caputo(x, .5) = d*x - Wc@x (over time) == G @ x_b with G = diag(d) - Wc,
    host-precomputed; becomes a single matmul per batch.
  * tanh(g) = 2*sigmoid(2g) - 1 with the g-gate rows pre-scaled by 2 on the
    host, so one Sigmoid activation covers all four gates.

Sharding: pure data parallelism over batch (64 -> 8 per core, 8 cores).
All weights replicated; scatter/gather on host.

The two LSTM layers' scans are software-pipelined: layer 1 lags layer 0 by
one WIN-step window. Inside each For_i iteration we interleave one step of
layer 0 (window w) with one step of layer 1 (window w-1), so each layer's
serial elementwise chain hides under the other layer's 64 matmul+ldweights
pairs; this also keeps the PE continuously busy (HAM stays un-throttled).
Layer 1's input projection xw1 = A1 @ h0 + b1 is produced per-window as a
PE-efficient bulk matmul (f=WIN*PB) straight into SBUF.

On-core layout (hidden-major):
  hT, cT  : (128 part = hidden%128, cols = kchunk*8 + b)   [4*8=32 cols]
  gatesT  : (128 part = gate%128,  cols = gchunk*8 + b)    [16*8=128 cols]
  gate chunks host-permuted to [i, f, o, g]; sigmoid covers all 128 cols.
"""

import math
from contextlib import ExitStack

import numpy as np
import ml_dtypes

import concourse.bass as bass
import concourse.tile as tile
from concourse.tile import add_dep_helper
from concourse import mybir
from concourse.bass import ds
from concourse.bass_utils import run_bass_kernel_spmd

AF = mybir.ActivationFunctionType
OP = mybir.AluOpType
F32 = mybir.dt.float32
F32R = mybir.dt.float32r
BF16 = mybir.dt.bfloat16

B, T, N = 64, 512, 250
NP = 256          # n padded to 2 partition chunks
H = 512
G4 = 4 * H        # 2048
OUT = 1024
NCORES = 8
PB = B // NCORES  # 8 batches per core
WIN = 64          # scan steps per For_i iteration
NWIN = T // WIN

KC = H // 128     # 4 hidden chunks
GC = G4 // 128    # 16 gate chunks
NC2 = NP // 128   # 2 input chunks
CB = KC * PB      # 32 h/c columns


def _split_drain_waits(nc, max_waits=1):
    """This walrus build's CoreV3 codegen accepts at most one sem-wait per
    engine instruction (Drain/Matmult/... ISA structs have a single wait
    slot).  Move extra waits onto same-engine NoOps inserted immediately
    before the instruction — the engine blocks at the NoOp instead, which is
    semantically identical (same engine stream, same program point)."""
    for bb in nc.m.functions[0].blocks:
        insts = bb.instructions  # live list
        i = 0
        while i < len(insts):
            ins = insts[i]
            si = ins.sync_info
            if si is not None and len(si.on_wait) > max_waits:
                waits = list(si.on_wait)
                ins.sync_info = mybir.SyncInfo(
                    on_wait=waits[:max_waits], on_update=list(si.on_update)
                )
                for j, w in enumerate(waits[max_waits:]):
                    nop = mybir.InstNoOp(name=f"{ins.name}-wsplit{j}")
                    nop.engine = ins.engine
                    nop.sync_info = mybir.SyncInfo(on_wait=[w], on_update=[])
                    insts.insert(i, nop)
                    i += 1
            i += 1


class _LayerState:
    """Per-layer persistent scan state + pools."""

    def __init__(self, nc, tc, ctx, name, whh_sb):
        self.name = name
        self.whh_sb = whh_sb
        pool = ctx.enter_context(tc.tile_pool(name=f"{name}_state", bufs=1))
        self.h_cur = pool.tile([128, CB], BF16)
        self.c_cur = pool.tile([128, CB], F32)
        nc.vector.memset(self.h_cur[:], 0.0)
        nc.vector.memset(self.c_cur[:], 0.0)
        self.ps_pool = ctx.enter_context(
            tc.tile_pool(name=f"{name}_ps", bufs=3, space="PSUM")
        )
        self.ew_pool = ctx.enter_context(tc.tile_pool(name=f"{name}_ew", bufs=3))
        self.sigma_dep = None  # scheduler-only ACT-order edge
        self.hw_pool = ctx.enter_context(tc.tile_pool(name=f"{name}_hw", bufs=3))


def _gen_window_steps(nc, st, id_sb, xw_u_view, hwin, bias_sb=None):
    """Generator emitting one LSTM step per next() for a WIN-step window.

    st       : _LayerState
    id_sb    : (128, 128) f32 identity; xw enters PSUM via id.T @ xw so the
               gates never take a DVE add (sigmoid reads PSUM directly)
    xw_u_view: callable u -> AP (128, GC, PB) input contribution for step u
    hwin     : SBUF tile (128, KC*WIN*PB) bf16 to dump h_t into, or None
    """
    h_prev = [st.h_cur[:, kc * PB:(kc + 1) * PB] for kc in range(KC)]
    c_prev = st.c_cur
    for u in range(WIN):
        psum = st.ps_pool.tile([128, GC * PB], F32, tag=f"{st.name}ps")
        # xw[t] into psum first — no h dependency, runs in the PE bubble
        nc.tensor.matmul(
            psum.rearrange("p (g b) -> p g b", g=GC),
            id_sb[:],
            xw_u_view(u),
            start=True,
            stop=False,
        )
        if bias_sb is not None:
            nc.tensor.matmul(
                psum[:, :], id_sb[:], bias_sb[:, :], start=False, stop=False,
            )
        for gc in range(GC):
            for kc in range(KC):
                nc.tensor.matmul(
                    psum[:, gc * PB:(gc + 1) * PB],
                    st.whh_sb[:, kc, gc * 128:(gc + 1) * 128],
                    h_prev[kc],
                    start=False,
                    stop=(gc == GC - 1 and kc == KC - 1),
                )
        # one sigmoid over all 128 cols; g rows were pre-scaled by 2 so
        # tanh(g) = 2*sigmoid(2g) - 1
        acts = st.ew_pool.tile([128, GC * PB], F32, tag=f"{st.name}a")
        sig = nc.scalar.activation(acts[:], psum[:], AF.Sigmoid)
        if st.sigma_dep is not None:
            # scheduler-only edge: keep ACT FIFO order sigma/tanh alternating
            # between the two layers so neither tanh is gated behind the
            # other layer's matmul-waiting sigma
            add_dep_helper(sig.ins, st.sigma_dep, sync=False,
                           reason="act-order")
        gp = st.ew_pool.tile([128, CB], F32, tag=f"{st.name}gp")
        nc.vector.tensor_scalar(
            gp[:], acts[:, 3 * CB:], 2.0, 1.0, OP.mult, OP.subtract
        )
        # c = f*c + i*g ; h = o*tanh(c)
        ig = st.ew_pool.tile([128, CB], F32, tag=f"{st.name}ig")
        nc.vector.tensor_tensor(ig[:], acts[:, :CB], gp[:], OP.mult)
        fc = st.ew_pool.tile([128, CB], F32, tag=f"{st.name}fc")
        nc.vector.tensor_tensor(fc[:], acts[:, CB:2 * CB], c_prev[:], OP.mult)
        c_new = (
            st.c_cur if u == WIN - 1
            else st.hw_pool.tile([128, CB], F32, tag=f"{st.name}c")
        )
        nc.vector.tensor_tensor(c_new[:], fc[:], ig[:], OP.add)
        tc_t = st.ew_pool.tile([128, CB], F32, tag=f"{st.name}tc")
        tanh_inst = nc.scalar.activation(tc_t[:], c_new[:], AF.Tanh)
        acts_o = acts[:, 2 * CB:3 * CB].rearrange("p (k b) -> p k b", k=KC)
        tc_v = tc_t[:].rearrange("p (k b) -> p k b", k=KC)
        if hwin is not None:
            h_out = hwin.rearrange("p (k w b) -> p w k b", k=KC, w=WIN)[:, u]
        elif u == WIN - 1:
            h_out = st.h_cur[:].rearrange("p (k b) -> p k b", k=KC)
        else:
            h_tmp = st.hw_pool.tile([128, CB], BF16, tag=f"{st.name}h")
            h_out = h_tmp[:].rearrange("p (k b) -> p k b", k=KC)
        nc.vector.tensor_tensor(h_out, acts_o, tc_v, OP.mult)
        if hwin is not None and u == WIN - 1:
            nc.vector.tensor_copy(
                st.h_cur[:].rearrange("p (k b) -> p k b", k=KC), h_out
            )
        h_prev = [h_out[:, kc, :] for kc in range(KC)]
        c_prev = c_new
        yield tanh_inst.ins


QW = 8  # bulk xw1 slice: 8 steps, f = QW*PB = 64


def _emit_bulk_xw1_quarter(nc, bulk_ps_pool, a1_sb, h0win, xw1sb, qi):
    """xw1 = A1 @ h0 for steps [qi*QW, (qi+1)*QW) of the current window
    (b1 is injected per-step by the bias id-matmul instead).

    Emitted right after L1's step qi*QW+QW-1 so the matmuls fill the PE
    bubble while L0's elementwise chain runs.  One 2-bank PSUM tile for
    all 16 gate chunks, drained by a single DVE copy."""
    qoff = qi * QW * PB
    psB = bulk_ps_pool.tile([128, GC * QW * PB], F32, tag="bps")
    for gc in range(GC):
        for kc in range(KC):
            nc.tensor.matmul(
                psB[:, gc * QW * PB:(gc + 1) * QW * PB],
                a1_sb[:, kc, gc * 128:(gc + 1) * 128],
                h0win[:, kc * WIN * PB + qoff:kc * WIN * PB + qoff + QW * PB],
                start=(kc == 0),
                stop=(kc == KC - 1),
            )
    dst = xw1sb[:].rearrange(
        "p (g w b) -> p g (w b)", g=GC, w=WIN
    )[:, :, qoff:qoff + QW * PB]
    nc.vector.tensor_copy(
        dst, psB[:].rearrange("p (g q) -> p g q", g=GC)
    )


def build_nc():
    nc = bass.Bass()

    x_in = nc.dram_tensor("x", [PB, T, NP], F32R, kind="ExternalInput")
    gt_in = nc.dram_tensor("gt", [KC, 128, T], F32R, kind="ExternalInput")
    a0_in = nc.dram_tensor("a0t", [NC2, 128, G4], BF16, kind="ExternalInput")
    b0_in = nc.dram_tensor("b0", [128, GC], F32, kind="ExternalInput")
    whh0_in = nc.dram_tensor("whh0t", [KC, 128, G4], BF16, kind="ExternalInput")
    a1_in = nc.dram_tensor("a1t", [KC, 128, G4], BF16, kind="ExternalInput")
    b1_in = nc.dram_tensor("b1", [128, GC], F32, kind="ExternalInput")
    whh1_in = nc.dram_tensor("whh1t", [KC, 128, G4], BF16, kind="ExternalInput")
    wout_in = nc.dram_tensor("woutt", [KC, 128, OUT], BF16, kind="ExternalInput")
    bout_in = nc.dram_tensor("boutr", [PB, OUT], F32, kind="ExternalInput")
    ident_in = nc.dram_tensor("ident", [128, 128], BF16, kind="ExternalInput")
    b1r_in = nc.dram_tensor("b1r", [128, GC * PB], BF16, kind="ExternalInput")
    out_ext = nc.dram_tensor("out", [PB, OUT], F32, kind="ExternalOutput")

    xw0_dram = nc.dram_tensor("xw0s", [GC, 128, PB, T], BF16)

    with tile.TileContext(nc) as tc:
        with ExitStack() as ctx:
            const_pool = ctx.enter_context(tc.tile_pool(name="consts", bufs=1))

            b0_sb = const_pool.tile([128, GC], F32)
            nc.sync.dma_start(b0_sb[:], b0_in[:, :])
            whh0_sb = const_pool.tile([128, KC, G4], BF16)
            nc.sync.dma_start(whh0_sb[:], whh0_in[:, :, :].rearrange("k p g -> p k g"))
            a1_sb = const_pool.tile([128, KC, G4], BF16)
            nc.sync.dma_start(a1_sb[:], a1_in[:, :, :].rearrange("k p g -> p k g"))
            b1_sb = const_pool.tile([128, GC], F32)
            nc.sync.dma_start(b1_sb[:], b1_in[:, :])
            whh1_sb = const_pool.tile([128, KC, G4], BF16)
            nc.sync.dma_start(whh1_sb[:], whh1_in[:, :, :].rearrange("k p g -> p k g"))
            id_sb = const_pool.tile([128, 128], BF16)
            nc.sync.dma_start(id_sb[:], ident_in[:, :])
            b1r_sb = const_pool.tile([128, GC * PB], BF16)
            nc.sync.dma_start(b1r_sb[:], b1r_in[:, :])

            # ---- phase A+B: featsT_b = x_bT @ G^T ; xw0 = A0 @ feats + b0 ----
            with tc.tile_pool(name="ab", bufs=2) as ab_pool, \
                 tc.tile_pool(name="abw", bufs=1) as abw_pool, \
                 tc.tile_pool(name="abf", bufs=1) as abf_pool, \
                 tc.tile_pool(name="abps", bufs=2, space="PSUM") as abps_pool:
                gt_sb = abw_pool.tile([128, KC, T], F32R)
                nc.sync.dma_start(
                    gt_sb[:], gt_in[:, :, :].rearrange("k p t -> p k t")
                )
                a0_sb = abw_pool.tile([128, NC2, G4], BF16)
                nc.sync.dma_start(
                    a0_sb[:], a0_in[:, :, :].rearrange("k p g -> p k g")
                )
                feats = []
                for b in range(PB):
                    x_sb = ab_pool.tile([128, KC, NP], F32R, tag="x")
                    nc.sync.dma_start(
                        x_sb[:], x_in[b].rearrange("(k p) n -> p k n", p=128)
                    )
                    fb = abf_pool.tile([128, NC2, T], BF16, tag=f"feats{b}")
                    for mc in range(NC2):
                        psA = abps_pool.tile([128, T], F32, tag="psA")
                        for kc in range(KC):
                            nc.tensor.matmul(
                                psA[:],
                                x_sb[:, kc, mc * 128:(mc + 1) * 128],
                                gt_sb[:, kc, :],
                                start=(kc == 0),
                                stop=(kc == KC - 1),
                            )
                        nc.vector.tensor_copy(fb[:, mc, :], psA[:])
                    feats.append(fb)
                dma_engs = [nc.sync, nc.scalar]
                for gc in range(GC):
                    for b in range(PB):
                        psB = abps_pool.tile([128, T], F32, tag="psB")
                        for kc in range(NC2):
                            nc.tensor.matmul(
                                psB[:],
                                a0_sb[:, kc, gc * 128:(gc + 1) * 128],
                                feats[b][:, kc, :],
                                start=(kc == 0),
                                stop=(kc == NC2 - 1),
                            )
                        xwb = ab_pool.tile([128, T], BF16, tag="xwb")
                        nc.scalar.activation(
                            xwb[:], psB[:], AF.Identity,
                            bias=b0_sb[:, gc:gc + 1],
                        )
                        dma_engs[b % 2].dma_start(
                            xw0_dram[gc, :, b, :], xwb[:]
                        )

            # ---- fused pipelined scan: L0(win w) + L1(win w-1) + bulk xw1(w) --
            st0 = _LayerState(nc, tc, ctx, "s0", whh0_sb)
            st1 = _LayerState(nc, tc, ctx, "s1", whh1_sb)
            pipe_pool = ctx.enter_context(tc.tile_pool(name="pipe", bufs=1))
            h0win = pipe_pool.tile([128, KC * WIN * PB], BF16)
            xw1sb = pipe_pool.tile([128, GC * WIN * PB], BF16)
            xw1_v = xw1sb[:].rearrange("p (g w b) -> p w g b", g=GC, w=WIN)

            with tc.tile_pool(name="win", bufs=1) as win_pool, \
                 tc.tile_pool(name="bulk_ps", bufs=1, space="PSUM") as bulk_ps_pool:

                def emit_win0(iw, run_l0, run_l1):
                    """One pipeline stage: L0 on window iw, L1 on window iw-1."""
                    if run_l0:
                        win = win_pool.tile([128, GC * PB * WIN], BF16, tag="win")
                        win_4d = win[:].rearrange(
                            "p (g b w) -> p g b w", g=GC, b=PB
                        )
                        wengs = [nc.sync, nc.scalar]
                        for bd in range(PB):
                            wengs[bd % 2].dma_start(
                                win_4d[:, :, bd, :],
                                xw0_dram[:, :, bd, ds(iw * WIN, WIN)]
                                .rearrange("g p w -> p g w"),
                            )
                        g0 = _gen_window_steps(
                            nc, st0, id_sb,
                            lambda u: win_4d[:, :, :, u:u + 1],
                            h0win,
                        )
                    g1 = None
                    if run_l1:
                        g1 = _gen_window_steps(
                            nc, st1, id_sb, lambda u: xw1_v[:, u], None,
                            bias_sb=b1r_sb,
                        )
                    prev_t1 = None
                    for u in range(WIN):
                        t0 = None
                        if run_l0:
                            st0.sigma_dep = prev_t1
                            t0 = next(g0)
                        if g1 is not None:
                            st1.sigma_dep = t0
                            prev_t1 = next(g1)
                        # quarter-bulk xw1 after L1 has consumed its old cols
                        if run_l0 and u % QW == QW - 1:
                            _emit_bulk_xw1_quarter(
                                nc, bulk_ps_pool, a1_sb, h0win, xw1sb, u // QW,
                            )

                emit_win0(0, True, False)            # peel: L0 window 0
                with tc.For_i(
                    1, NWIN, 1, hint_engines=(mybir.EngineType.PE,)
                ) as iw:
                    emit_win0(iw, True, True)        # L0 win iw, L1 win iw-1
                emit_win0(NWIN, False, True)         # peel: L1 window 15

            # ---- phase F: out = relu(h1_last @ Wout.T + bout) ----
            with tc.tile_pool(name="f_ps", bufs=2, space="PSUM") as fps_pool, \
                 tc.tile_pool(name="f_o", bufs=1) as fo_pool:
                wout_sb = fo_pool.tile([128, KC, OUT], BF16)
                nc.sync.dma_start(
                    wout_sb[:], wout_in[:, :, :].rearrange("k p g -> p k g")
                )
                bout_sb = fo_pool.tile([PB, OUT], F32)
                nc.sync.dma_start(bout_sb[:], bout_in[:, :])
                out_sb = bout_sb
                for half in range(2):
                    psF = fps_pool.tile([PB, 512], F32, tag="psF")
                    for kc in range(KC):
                        nc.tensor.matmul(
                            psF[:],
                            st1.h_cur[:, kc * PB:(kc + 1) * PB],
                            wout_sb[:, kc, half * 512:(half + 1) * 512],
                            start=(kc == 0),
                            stop=(kc == KC - 1),
                        )
                    sl = slice(half * 512, (half + 1) * 512)
                    nc.vector.tensor_tensor(
                        out_sb[:, sl], psF[:], bout_sb[:, sl], OP.add
                    )
                    nc.vector.tensor_scalar_max(out_sb[:, sl], out_sb[:, sl], 0.0)
                nc.sync.dma_start(out_ext[:, :], out_sb[:])

    _split_drain_waits(nc)
    return nc


_NC_CACHE = None


def _get_nc():
    global _NC_CACHE
    if _NC_CACHE is None:
        _NC_CACHE = build_nc()
    return _NC_CACHE


def _prep_host(inputs):
    x = np.asarray(inputs["x"], dtype=np.float32)
    coef = 1.0 / math.gamma(0.5)
    t = np.arange(T, dtype=np.float64)
    diff = t[:, None] - t[None, :]
    W = np.where(diff > 0, (np.abs(diff) + 1e-6) ** -0.5, 0.0).astype(np.float32)
    d = (coef * W.sum(1)).astype(np.float32)
    G = (np.diag(d) - coef * W).astype(np.float32)  # feats_b = G @ x_b
    GT = np.ascontiguousarray(G.T).reshape(KC, 128, T)

    perm = np.concatenate([  # torch gate order i,f,g,o -> [i,f,o,g]
        np.arange(0, H), np.arange(H, 2 * H),
        np.arange(3 * H, 4 * H), np.arange(2 * H, 3 * H),
    ])
    # g rows scaled by 2: tanh(g) computed on-chip as 2*sigmoid(2g)-1
    gscale = np.ones((G4, 1), np.float32)
    gscale[3 * H:] = 2.0
    bf = ml_dtypes.bfloat16

    A0 = np.zeros((G4, NP), np.float32)
    A0[:, :N] = np.asarray(inputs["Wih0"], np.float32)[perm, :N] * gscale
    A0T = np.ascontiguousarray(A0.T).astype(bf).reshape(NC2, 128, G4)
    b0 = ((np.asarray(inputs["bih0"], np.float32)
           + np.asarray(inputs["bhh0"], np.float32))[perm] * gscale[:, 0])
    b0_t = np.ascontiguousarray(b0.reshape(GC, 128).T)
    Whh0T = np.ascontiguousarray(
        (np.asarray(inputs["Whh0"], np.float32)[perm] * gscale).T
    ).astype(bf).reshape(KC, 128, G4)

    A1T = np.ascontiguousarray(
        (np.asarray(inputs["Wih1"], np.float32)[perm] * gscale).T
    ).astype(bf).reshape(KC, 128, G4)
    b1 = ((np.asarray(inputs["bih1"], np.float32)
           + np.asarray(inputs["bhh1"], np.float32))[perm] * gscale[:, 0])
    b1_t = np.ascontiguousarray(b1.reshape(GC, 128).T)
    Whh1T = np.ascontiguousarray(
        (np.asarray(inputs["Whh1"], np.float32)[perm] * gscale).T
    ).astype(bf).reshape(KC, 128, G4)

    WoutT = np.ascontiguousarray(
        np.asarray(inputs["Wout"], np.float32).T
    ).astype(bf).reshape(KC, 128, OUT)
    bout_r = np.broadcast_to(
        np.asarray(inputs["bout"], np.float32), (PB, OUT)
    ).copy()

    xp = np.zeros((B, T, NP), np.float32)
    xp[:, :, :N] = x

    b1r = np.repeat(b1_t[:, :, None], PB, axis=2).reshape(128, GC * PB)
    shared = dict(
        gt=GT, a0t=A0T, b0=b0_t, whh0t=Whh0T, a1t=A1T, b1=b1_t,
        whh1t=Whh1T, woutt=WoutT, boutr=bout_r,
        ident=np.eye(128).astype(ml_dtypes.bfloat16),
        b1r=b1r.astype(ml_dtypes.bfloat16),
    )
    in_maps = []
    for c in range(NCORES):
        m = dict(shared)
        m["x"] = np.ascontiguousarray(xp[c * PB:(c + 1) * PB])
        in_maps.append(m)
    return in_maps


def kernel(**inputs):
    nc = _get_nc()
    in_maps = _prep_host(inputs)
    res = run_bass_kernel_spmd(nc, in_maps, core_ids=list(range(NCORES)))
    out = np.concatenate([r["out"] for r in res.results], axis=0)
    return out.astype(np.float32)


# revision 25
# speedup vs baseline: 1.0819x; 1.0045x over previous
"""Trainium2 Bass kernel for the CaputoEncoder model.

Model (see reference): feats = concat([caputo(x, 0.5), caputo(x, 1.0)], -1)
-> 2-layer LSTM(512) -> last timestep -> relu(linear).

Key simplifications:
  * caputo(x, 1.0) has coefficient 1/gamma(0) == 0 -> contributes zeros;
    only the alpha=0.5 branch matters, so only Wih0[:, :250] is ever used.
  * caputo(x, .5) = d*x - Wc@x (over time) == G @ x_b with G = diag(d) - Wc,
    host-precomputed; becomes a single matmul per batch.
  * tanh(g) = 2*sigmoid(2g) - 1 with the g-gate rows pre-scaled by 2 on the
    host, so one Sigmoid activation covers all four gates.

Sharding: pure data parallelism over batch (64 -> 8 per core, 8 cores).
All weights replicated; scatter/gather on host.

The two LSTM layers' scans are software-pipelined: layer 1 lags layer 0 by
one WIN-step window. Inside each For_i iteration we interleave one step of
layer 0 (window w) with one step of layer 1 (window w-1), so each layer's
serial elementwise chain hides under the other layer's 64 matmul+ldweights
pairs; this also keeps the PE continuously busy (HAM stays un-throttled).
Layer 1's input projection xw1 = A1 @ h0 + b1 is produced per-window as a
PE-efficient bulk matmul (f=WIN*PB) straight into SBUF.

On-core layout (hidden-major):
  hT, cT  : (128 part = hidden%128, cols = kchunk*8 + b)   [4*8=32 cols]
  gatesT  : (128 part = gate%128,  cols = gchunk*8 + b)    [16*8=128 cols]
  gate chunks host-permuted to [i, f, o, g]; sigmoid covers all 128 cols.
"""

import math
from contextlib import ExitStack

import numpy as np
import ml_dtypes

import concourse.bass as bass
import concourse.tile as tile
from concourse.tile import add_dep_helper
from concourse import mybir
from concourse.bass import ds
from concourse.bass_utils import run_bass_kernel_spmd

AF = mybir.ActivationFunctionType
OP = mybir.AluOpType
F32 = mybir.dt.float32
F32R = mybir.dt.float32r
BF16 = mybir.dt.bfloat16

B, T, N = 64, 512, 250
NP = 256          # n padded to 2 partition chunks
H = 512
G4 = 4 * H        # 2048
OUT = 1024
NCORES = 8
PB = B // NCORES  # 8 batches per core
WIN = 64          # scan steps per For_i iteration
NWIN = T // WIN

KC = H // 128     # 4 hidden chunks
GC = G4 // 128    # 16 gate chunks
NC2 = NP // 128   # 2 input chunks
CB = KC * PB      # 32 h/c columns


def _split_drain_waits(nc, max_waits=1):
    """This walrus build's CoreV3 codegen accepts at most one sem-wait per
    engine instruction (Drain/Matmult/... ISA structs have a single wait
    slot).  Move extra waits onto same-engine NoOps inserted immediately
    before the instruction — the engine blocks at the NoOp instead, which is
    semantically identical (same engine stream, same program point)."""
    for bb in nc.m.functions[0].blocks:
        insts = bb.instructions  # live list
        i = 0
        while i < len(insts):
            ins = insts[i]
            si = ins.sync_info
            if si is not None and len(si.on_wait) > max_waits:
                waits = list(si.on_wait)
                ins.sync_info = mybir.SyncInfo(
                    on_wait=waits[:max_waits], on_update=list(si.on_update)
                )
                for j, w in enumerate(waits[max_waits:]):
                    nop = mybir.InstNoOp(name=f"{ins.name}-wsplit{j}")
                    nop.engine = ins.engine
                    nop.sync_info = mybir.SyncInfo(on_wait=[w], on_update=[])
                    insts.insert(i, nop)
                    i += 1
            i += 1


class _LayerState:
    """Per-layer persistent scan state + pools."""

    def __init__(self, nc, tc, ctx, name, whh_sb):
        self.name = name
        self.whh_sb = whh_sb
        pool = ctx.enter_context(tc.tile_pool(name=f"{name}_state", bufs=1))
        self.h_cur = pool.tile([128, CB], BF16)
        self.c_cur = pool.tile([128, CB], F32)
        nc.vector.memset(self.h_cur[:], 0.0)
        nc.vector.memset(self.c_cur[:], 0.0)
        self.ps_pool = ctx.enter_context(
            tc.tile_pool(name=f"{name}_ps", bufs=3, space="PSUM")
        )
        self.ew_pool = ctx.enter_context(tc.tile_pool(name=f"{name}_ew", bufs=3))
        self.sigma_dep = None  # scheduler-only ACT-order edge
        self.hw_pool = ctx.enter_context(tc.tile_pool(name=f"{name}_hw", bufs=3))


def _gen_window_steps(nc, st, id_sb, xw_u_view, hwin, bias_sb=None):
    """Generator emitting one LSTM step per next() for a WIN-step window.

    st       : _LayerState
    id_sb    : (128, 128) f32 identity; xw enters PSUM via id.T @ xw so the
               gates never take a DVE add (sigmoid reads PSUM directly)
    xw_u_view: callable u -> AP (128, GC, PB) input contribution for step u
    hwin     : SBUF tile (128, KC*WIN*PB) bf16 to dump h_t into, or None
    """
    h_prev = [st.h_cur[:, kc * PB:(kc + 1) * PB] for kc in range(KC)]
    c_prev = st.c_cur
    for u in range(WIN):
        psum = st.ps_pool.tile([128, GC * PB], F32, tag=f"{st.name}ps")
        # xw[t] into psum first — no h dependency, runs in the PE bubble
        nc.tensor.matmul(
            psum.rearrange("p (g b) -> p g b", g=GC),
            id_sb[:],
            xw_u_view(u),
            start=True,
            stop=False,
        )
        if bias_sb is not None:
            nc.tensor.matmul(
                psum[:, :], id_sb[:], bias_sb[:, :], start=False, stop=False,
            )
        for gc in range(GC):
            for kc in range(KC):
                nc.tensor.matmul(
                    psum[:, gc * PB:(gc + 1) * PB],
                    st.whh_sb[:, kc, gc * 128:(gc + 1) * 128],
                    h_prev[kc],
                    start=False,
                    stop=(gc == GC - 1 and kc == KC - 1),
                )
        # one sigmoid over all 128 cols; g rows were pre-scaled by 2 so
        # tanh(g) = 2*sigmoid(2g) - 1
        acts = st.ew_pool.tile([128, GC * PB], F32, tag=f"{st.name}a")
        sig = nc.scalar.activation(acts[:], psum[:], AF.Sigmoid)
        if st.sigma_dep is not None:
            # scheduler-only edge: keep ACT FIFO order sigma/tanh alternating
            # between the two layers so neither tanh is gated behind the
            # other layer's matmul-waiting sigma
            add_dep_helper(sig.ins, st.sigma_dep, sync=False,
                           reason="act-order")
        gp = st.ew_pool.tile([128, CB], F32, tag=f"{st.name}gp")
        nc.vector.tensor_scalar(
            gp[:], acts[:, 3 * CB:], 2.0, 1.0, OP.mult, OP.subtract
        )
        # c = f*c + i*g ; h = o*tanh(c)
        ig = st.ew_pool.tile([128, CB], F32, tag=f"{st.name}ig")
        nc.vector.tensor_tensor(ig[:], acts[:, :CB], gp[:], OP.mult)
        fc = st.ew_pool.tile([128, CB], F32, tag=f"{st.name}fc")
        nc.vector.tensor_tensor(fc[:], acts[:, CB:2 * CB], c_prev[:], OP.mult)
        c_new = (
            st.c_cur if u == WIN - 1
            else st.hw_pool.tile([128, CB], F32, tag=f"{st.name}c")
        )
        nc.vector.tensor_tensor(c_new[:], fc[:], ig[:], OP.add)
        tc_t = st.ew_pool.tile([128, CB], F32, tag=f"{st.name}tc")
        tanh_inst = nc.scalar.activation(tc_t[:], c_new[:], AF.Tanh)
        acts_o = acts[:, 2 * CB:3 * CB].rearrange("p (k b) -> p k b", k=KC)
        tc_v = tc_t[:].rearrange("p (k b) -> p k b", k=KC)
        if hwin is not None:
            h_out = hwin.rearrange("p (k w b) -> p w k b", k=KC, w=WIN)[:, u]
        elif u == WIN - 1:
            h_out = st.h_cur[:].rearrange("p (k b) -> p k b", k=KC)
        else:
            h_tmp = st.hw_pool.tile([128, CB], BF16, tag=f"{st.name}h")
            h_out = h_tmp[:].rearrange("p (k b) -> p k b", k=KC)
        nc.vector.tensor_tensor(h_out, acts_o, tc_v, OP.mult)
        if hwin is not None and u == WIN - 1:
            nc.vector.tensor_copy(
                st.h_cur[:].rearrange("p (k b) -> p k b", k=KC), h_out
            )
        h_prev = [h_out[:, kc, :] for kc in range(KC)]
        c_prev = c_new
        yield tanh_inst.ins


QW = 8  # bulk xw1 slice: 8 steps, f = QW*PB = 64


def _emit_bulk_xw1_quarter(nc, bulk_ps_pool, a1_sb, h0win, xw1sb, qi):
    """xw1 = A1 @ h0 for steps [qi*QW, (qi+1)*QW) of the current window
    (b1 is injected per-step by the bias id-matmul instead).

    Emitted right after L1's step qi*QW+QW-1 so the matmuls fill the PE
    bubble while L0's elementwise chain runs.  One 2-bank PSUM tile for
    all 16 gate chunks, drained by a single DVE copy."""
    qoff = qi * QW * PB
    psB = bulk_ps_pool.tile([128, GC * QW * PB], F32, tag="bps")
    for gc in range(GC):
        for kc in range(KC):
            nc.tensor.matmul(
                psB[:, gc * QW * PB:(gc + 1) * QW * PB],
                a1_sb[:, kc, gc * 128:(gc + 1) * 128],
                h0win[:, kc * WIN * PB + qoff:kc * WIN * PB + qoff + QW * PB],
                start=(kc == 0),
                stop=(kc == KC - 1),
            )
    dst = xw1sb[:].rearrange(
        "p (g w b) -> p g (w b)", g=GC, w=WIN
    )[:, :, qoff:qoff + QW * PB]
    nc.vector.tensor_copy(
        dst, psB[:].rearrange("p (g q) -> p g q", g=GC)
    )


def build_nc():
    nc = bass.Bass()

    x_in = nc.dram_tensor("x", [PB, T, NP], F32R, kind="ExternalInput")
    gt_in = nc.dram_tensor("gt", [KC, 128, T], F32R, kind="ExternalInput")
    a0_in = nc.dram_tensor("a0t", [NC2, 128, G4], BF16, kind="ExternalInput")
    b0_in = nc.dram_tensor("b0", [128, GC], F32, kind="ExternalInput")
    whh0_in = nc.dram_tensor("whh0t", [KC, 128, G4], BF16, kind="ExternalInput")
    a1_in = nc.dram_tensor("a1t", [KC, 128, G4], BF16, kind="ExternalInput")
    b1_in = nc.dram_tensor("b1", [128, GC], F32, kind="ExternalInput")
    whh1_in = nc.dram_tensor("whh1t", [KC, 128, G4], BF16, kind="ExternalInput")
    wout_in = nc.dram_tensor("woutt", [KC, 128, OUT], BF16, kind="ExternalInput")
    bout_in = nc.dram_tensor("boutr", [PB, OUT], F32, kind="ExternalInput")
    ident_in = nc.dram_tensor("ident", [128, 128], BF16, kind="ExternalInput")
    b1r_in = nc.dram_tensor("b1r", [128, GC * PB], BF16, kind="ExternalInput")
    out_ext = nc.dram_tensor("out", [PB, OUT], F32, kind="ExternalOutput")

    xw0_dram = nc.dram_tensor("xw0s", [GC, 128, PB, T], BF16)

    with tile.TileContext(nc) as tc:
        with ExitStack() as ctx:
            const_pool = ctx.enter_context(tc.tile_pool(name="consts", bufs=1))

            b0_sb = const_pool.tile([128, GC], F32)
            nc.sync.dma_start(b0_sb[:], b0_in[:, :])
            whh0_sb = const_pool.tile([128, KC, G4], BF16)
            nc.sync.dma_start(whh0_sb[:], whh0_in[:, :, :].rearrange("k p g -> p k g"))
            a1_sb = const_pool.tile([128, KC, G4], BF16)
            nc.sync.dma_start(a1_sb[:], a1_in[:, :, :].rearrange("k p g -> p k g"))
            b1_sb = const_pool.tile([128, GC], F32)
            nc.sync.dma_start(b1_sb[:], b1_in[:, :])
            whh1_sb = const_pool.tile([128, KC, G4], BF16)
            nc.sync.dma_start(whh1_sb[:], whh1_in[:, :, :].rearrange("k p g -> p k g"))
            id_sb = const_pool.tile([128, 128], BF16)
            nc.sync.dma_start(id_sb[:], ident_in[:, :])
            b1r_sb = const_pool.tile([128, GC * PB], BF16)
            nc.sync.dma_start(b1r_sb[:], b1r_in[:, :])

            # ---- phase A+B: featsT_b = x_bT @ G^T ; xw0 = A0 @ feats + b0 ----
            with tc.tile_pool(name="ab", bufs=2) as ab_pool, \
                 tc.tile_pool(name="abw", bufs=1) as abw_pool, \
                 tc.tile_pool(name="abf", bufs=1) as abf_pool, \
                 tc.tile_pool(name="abps", bufs=2, space="PSUM") as abps_pool:
                gt_sb = abw_pool.tile([128, KC, T], F32R)
                nc.sync.dma_start(
                    gt_sb[:], gt_in[:, :, :].rearrange("k p t -> p k t")
                )
                a0_sb = abw_pool.tile([128, NC2, G4], BF16)
                nc.sync.dma_start(
                    a0_sb[:], a0_in[:, :, :].rearrange("k p g -> p k g")
                )
                feats = []
                for b in range(PB):
                    x_sb = ab_pool.tile([128, KC, NP], F32R, tag="x")
                    nc.sync.dma_start(
                        x_sb[:], x_in[b].rearrange("(k p) n -> p k n", p=128)
                    )
                    fb = abf_pool.tile([128, NC2, T], BF16, tag=f"feats{b}")
                    for mc in range(NC2):
                        psA = abps_pool.tile([128, T], F32, tag="psA")
                        for kc in range(KC):
                            nc.tensor.matmul(
                                psA[:],
                                x_sb[:, kc, mc * 128:(mc + 1) * 128],
                                gt_sb[:, kc, :],
                                start=(kc == 0),
                                stop=(kc == KC - 1),
                            )
                        nc.vector.tensor_copy(fb[:, mc, :], psA[:])
                    feats.append(fb)
                dma_engs = [nc.sync, nc.scalar]
                # first chunk covers scan window 0 so the peeled window's
                # (static-AP) win DMA can start while the rest of phase B
                # still runs
                for ci, (t0, tn) in enumerate([(0, WIN), (WIN, T - WIN)]):
                    for gc in range(GC):
                        for b in range(PB):
                            psB = abps_pool.tile(
                                [128, tn], F32, tag=f"psB{ci}"
                            )
                            for kc in range(NC2):
                                nc.tensor.matmul(
                                    psB[:],
                                    a0_sb[:, kc, gc * 128:(gc + 1) * 128],
                                    feats[b][:, kc, t0:t0 + tn],
                                    start=(kc == 0),
                                    stop=(kc == NC2 - 1),
                                )
                            xwb = ab_pool.tile(
                                [128, tn], BF16, tag=f"xwb{ci}"
                            )
                            nc.scalar.activation(
                                xwb[:], psB[:], AF.Identity,
                                bias=b0_sb[:, gc:gc + 1],
                            )
                            dma_engs[b % 2].dma_start(
                                xw0_dram[gc, :, b, t0:t0 + tn], xwb[:]
                            )

            # ---- fused pipelined scan: L0(win w) + L1(win w-1) + bulk xw1(w) --
            st0 = _LayerState(nc, tc, ctx, "s0", whh0_sb)
            st1 = _LayerState(nc, tc, ctx, "s1", whh1_sb)
            pipe_pool = ctx.enter_context(tc.tile_pool(name="pipe", bufs=1))
            h0win = pipe_pool.tile([128, KC * WIN * PB], BF16)
            xw1sb = pipe_pool.tile([128, GC * WIN * PB], BF16)
            xw1_v = xw1sb[:].rearrange("p (g w b) -> p w g b", g=GC, w=WIN)

            with tc.tile_pool(name="win", bufs=1) as win_pool, \
                 tc.tile_pool(name="bulk_ps", bufs=1, space="PSUM") as bulk_ps_pool:

                def emit_win0(iw, run_l0, run_l1):
                    """One pipeline stage: L0 on window iw, L1 on window iw-1."""
                    if run_l0:
                        win = win_pool.tile([128, GC * PB * WIN], BF16, tag="win")
                        win_4d = win[:].rearrange(
                            "p (g b w) -> p g b w", g=GC, b=PB
                        )
                        wengs = [nc.sync, nc.scalar]
                        for bd in range(PB):
                            wengs[bd % 2].dma_start(
                                win_4d[:, :, bd, :],
                                xw0_dram[:, :, bd, ds(iw * WIN, WIN)]
                                .rearrange("g p w -> p g w"),
                            )
                        g0 = _gen_window_steps(
                            nc, st0, id_sb,
                            lambda u: win_4d[:, :, :, u:u + 1],
                            h0win,
                        )
                    g1 = None
                    if run_l1:
                        g1 = _gen_window_steps(
                            nc, st1, id_sb, lambda u: xw1_v[:, u], None,
                            bias_sb=b1r_sb,
                        )
                    prev_t1 = None
                    for u in range(WIN):
                        t0 = None
                        if run_l0:
                            st0.sigma_dep = prev_t1
                            t0 = next(g0)
                        if g1 is not None:
                            st1.sigma_dep = t0
                            prev_t1 = next(g1)
                        # quarter-bulk xw1 after L1 has consumed its old cols
                        if run_l0 and u % QW == QW - 1:
                            _emit_bulk_xw1_quarter(
                                nc, bulk_ps_pool, a1_sb, h0win, xw1sb, u // QW,
                            )

                emit_win0(0, True, False)            # peel: L0 window 0
                with tc.For_i(
                    1, NWIN, 1, hint_engines=(mybir.EngineType.PE,)
                ) as iw:
                    emit_win0(iw, True, True)        # L0 win iw, L1 win iw-1
                emit_win0(NWIN, False, True)         # peel: L1 window 15

            # ---- phase F: out = relu(h1_last @ Wout.T + bout) ----
            with tc.tile_pool(name="f_ps", bufs=2, space="PSUM") as fps_pool, \
                 tc.tile_pool(name="f_o", bufs=1) as fo_pool:
                wout_sb = fo_pool.tile([128, KC, OUT], BF16)
                nc.sync.dma_start(
                    wout_sb[:], wout_in[:, :, :].rearrange("k p g -> p k g")
                )
                bout_sb = fo_pool.tile([PB, OUT], F32)
                nc.sync.dma_start(bout_sb[:], bout_in[:, :])
                out_sb = bout_sb
                for half in range(2):
                    psF = fps_pool.tile([PB, 512], F32, tag="psF")
                    for kc in range(KC):
                        nc.tensor.matmul(
                            psF[:],
                            st1.h_cur[:, kc * PB:(kc + 1) * PB],
                            wout_sb[:, kc, half * 512:(half + 1) * 512],
                            start=(kc == 0),
                            stop=(kc == KC - 1),
                        )
                    sl = slice(half * 512, (half + 1) * 512)
                    nc.vector.tensor_tensor(
                        out_sb[:, sl], psF[:], bout_sb[:, sl], OP.add
                    )
                    nc.vector.tensor_scalar_max(out_sb[:, sl], out_sb[:, sl], 0.0)
                nc.sync.dma_start(out_ext[:, :], out_sb[:])

    _split_drain_waits(nc)
    return nc


_NC_CACHE = None


def _get_nc():
    global _NC_CACHE
    if _NC_CACHE is None:
        _NC_CACHE = build_nc()
    return _NC_CACHE


def _prep_host(inputs):
    x = np.asarray(inputs["x"], dtype=np.float32)
    coef = 1.0 / math.gamma(0.5)
    t = np.arange(T, dtype=np.float64)
    diff = t[:, None] - t[None, :]
    W = np.where(diff > 0, (np.abs(diff) + 1e-6) ** -0.5, 0.0).astype(np.float32)
    d = (coef * W.sum(1)).astype(np.float32)
    G = (np.diag(d) - coef * W).astype(np.float32)  # feats_b = G @ x_b
    GT = np.ascontiguousarray(G.T).reshape(KC, 128, T)

    perm = np.concatenate([  # torch gate order i,f,g,o -> [i,f,o,g]
        np.arange(0, H), np.arange(H, 2 * H),
        np.arange(3 * H, 4 * H), np.arange(2 * H, 3 * H),
    ])
    # g rows scaled by 2: tanh(g) computed on-chip as 2*sigmoid(2g)-1
    gscale = np.ones((G4, 1), np.float32)
    gscale[3 * H:] = 2.0
    bf = ml_dtypes.bfloat16

    A0 = np.zeros((G4, NP), np.float32)
    A0[:, :N] = np.asarray(inputs["Wih0"], np.float32)[perm, :N] * gscale
    A0T = np.ascontiguousarray(A0.T).astype(bf).reshape(NC2, 128, G4)
    b0 = ((np.asarray(inputs["bih0"], np.float32)
           + np.asarray(inputs["bhh0"], np.float32))[perm] * gscale[:, 0])
    b0_t = np.ascontiguousarray(b0.reshape(GC, 128).T)
    Whh0T = np.ascontiguousarray(
        (np.asarray(inputs["Whh0"], np.float32)[perm] * gscale).T
    ).astype(bf).reshape(KC, 128, G4)

    A1T = np.ascontiguousarray(
        (np.asarray(inputs["Wih1"], np.float32)[perm] * gscale).T
    ).astype(bf).reshape(KC, 128, G4)
    b1 = ((np.asarray(inputs["bih1"], np.float32)
           + np.asarray(inputs["bhh1"], np.float32))[perm] * gscale[:, 0])
    b1_t = np.ascontiguousarray(b1.reshape(GC, 128).T)
    Whh1T = np.ascontiguousarray(
        (np.asarray(inputs["Whh1"], np.float32)[perm] * gscale).T
    ).astype(bf).reshape(KC, 128, G4)

    WoutT = np.ascontiguousarray(
        np.asarray(inputs["Wout"], np.float32).T
    ).astype(bf).reshape(KC, 128, OUT)
    bout_r = np.broadcast_to(
        np.asarray(inputs["bout"], np.float32), (PB, OUT)
    ).copy()

    xp = np.zeros((B, T, NP), np.float32)
    xp[:, :, :N] = x

    b1r = np.repeat(b1_t[:, :, None], PB, axis=2).reshape(128, GC * PB)
    shared = dict(
        gt=GT, a0t=A0T, b0=b0_t, whh0t=Whh0T, a1t=A1T, b1=b1_t,
        whh1t=Whh1T, woutt=WoutT, boutr=bout_r,
        ident=np.eye(128).astype(ml_dtypes.bfloat16),
        b1r=b1r.astype(ml_dtypes.bfloat16),
    )
    in_maps = []
    for c in range(NCORES):
        m = dict(shared)
        m["x"] = np.ascontiguousarray(xp[c * PB:(c + 1) * PB])
        in_maps.append(m)
    return in_maps


def kernel(**inputs):
    nc = _get_nc()
    in_maps = _prep_host(inputs)
    res = run_bass_kernel_spmd(nc, in_maps, core_ids=list(range(NCORES)))
    out = np.concatenate([r["out"] for r in res.results], axis=0)
    return out.astype(np.float32)
